# revision 1
# baseline (speedup 1.0000x reference)
"""Causal self-attention for trn2, 8 NeuronCores.

Problem: x[4,2048,1024] @ w_qkv[1024,3072] -> causal MHA (16 heads, d=64)
-> @ w_out[1024,1024].

Sharding: core c handles batch b=c%4 and heads hbase=8*(c//4)..hbase+8
(data parallel on B x tensor parallel on heads). Each core computes the
partial out-projection y_c = att_slice @ w_out[slice]; the host sums the
two partials per batch.

v4: all matmul operands bf16 (fp32 PSUM accumulation). x is cast to a
ct-major bf16 DRAM scratch (SWDGE cast-DMA, contiguous [2048,128] blocks)
and transposed with hardware DMA-transpose loads. All weights are cast
once into resident bf16 tiles by SWDGE cast-DMAs. Softmax denominators
come from a fused ones-column in the AV matmul ([V|1]^T w^T row 64);
causal masking skips above-diagonal tiles and applies one gpsimd
affine_select per diagonal 128x128 block after the exp. Normalization:
DVE reciprocal + DRAM-bounce partition broadcast + multiply, staged off
PSUM so nothing blocks the accumulators.

4-round pipeline over T-quarters: round r transposes quarter r, projects
qT/kT/V for it, runs attention q-block r for every head (causality needs
only k/V quarters <= r), then the out-projection for those q rows. PSUM:
sA/sB double-buffered [128,512] scores, av_A/av_B accumulators, and a
dedicated [128,1024] projection tag so next-round projection matmuls can
fill TensorE gaps while ScalarE paces the attention exps.
"""

import sys

for p in ("/opt/trn_rl_repo", "/opt/pypackages"):
    if p not in sys.path:
        sys.path.insert(0, p)

import contextlib

import numpy as np

import concourse.bass as bass
import concourse.mybir as mybir
import concourse.tile as tile
from concourse import bacc
from concourse.bass_utils import run_bass_kernel_spmd
from concourse.masks import make_identity

F32 = mybir.dt.float32
BF = mybir.dt.bfloat16
EXP = mybir.ActivationFunctionType.Exp

T = 2048          # sequence length
C = 1024          # model dim
HC = 8            # heads per core
D = 64            # head dim
NG = 4            # head-groups of 2 per core
NCT = C // 128    # 8 contraction tiles
NTT = T // 128    # 16 token tiles
SCALE = 0.125     # 1/sqrt(D)


def build_nc():
    nc = bacc.Bacc("TRN2", target_bir_lowering=False, debug=False)

    x_d = nc.dram_tensor("x", [T, C], F32, kind="ExternalInput")
    wq_d = nc.dram_tensor("wq", [C, 512], F32, kind="ExternalInput")
    wk_d = nc.dram_tensor("wk", [C, 512], F32, kind="ExternalInput")
    wv_d = nc.dram_tensor("wv", [C, 512], F32, kind="ExternalInput")
    wo_d = nc.dram_tensor("wo", [512, C], F32, kind="ExternalInput")
    y_d = nc.dram_tensor("y", [T, C], F32, kind="ExternalOutput")

    with tile.TileContext(nc) as tc, contextlib.ExitStack() as ctx:
        persist = ctx.enter_context(tc.tile_pool(name="persist", bufs=1))
        work = ctx.enter_context(tc.tile_pool(name="work", bufs=1))
        ps = ctx.enter_context(tc.tile_pool(name="ps", bufs=1, space="PSUM"))
        dpool = ctx.enter_context(tc.tile_pool(name="dram", bufs=1, space="DRAM"))

        kT = [persist.tile([128, T], BF, tag=f"kT{g}", name=f"kT{g}")
              for g in range(NG)]
        V = persist.tile([128, NTT, HC, 65], BF, tag="V")

        # x -> bf16 DRAM scratch. The cast must be a CONTIGUOUS SWDGE DMA:
        # strided cast-DMAs truncate instead of round-to-nearest, and the
        # truncation bias blows up the dot products downstream.
        xbf = dpool.tile([T, C], BF, tag="xbf", name="xbf")
        # round 0's xT comes from on-chip PE transposes so TensorE starts
        # within ~10us instead of waiting for the cast->DMA-transpose chain;
        # rounds 1-3 still use the cheap hardware DMA-transpose path.
        ident = persist.tile([128, 128], F32, tag="ident", name="ident")
        make_identity(nc, ident)
        xTq0 = [work.tile([128, 512], BF, tag=f"xTq{ct}", name=f"xTq{ct}",
                          bufs=2)
                for ct in range(NCT)]
        for j in range(4):
            x_nat = work.tile([128, C], F32, tag="x_nat", bufs=2, name="x_nat")
            nc.sync.dma_start(out=x_nat, in_=x_d.ap()[j * 128:(j + 1) * 128, :])
            tp0 = ps.tile([128, 1024], F32, tag="sc", bufs=2, name="tp0")
            for ct in range(NCT):
                nc.tensor.transpose(
                    tp0[:, ct * 128:(ct + 1) * 128],
                    x_nat[:, ct * 128:(ct + 1) * 128],
                    ident,
                )
            for ct in range(NCT):
                nc.vector.tensor_copy(
                    xTq0[ct][:, j * 128:(j + 1) * 128],
                    tp0[:, ct * 128:(ct + 1) * 128],
                )
        # qkv weights: direct f32 loads + DVE casts so round-0 projection
        # is never stuck behind the SWDGE cast chain; wo (needed latest)
        # keeps the DRAM-bounce cast.
        wq_bf = persist.tile([128, NCT, 512], BF, tag="wq_bf")
        wk_bf = persist.tile([128, NCT, 512], BF, tag="wk_bf")
        wv_bf = persist.tile([128, NCT, 512], BF, tag="wv_bf")
        for wdram, wbf in ((wq_d, wq_bf), (wk_d, wk_bf), (wv_d, wv_bf)):
            wstage = work.tile([128, NCT, 512], F32, tag="wstage", name="wstage")
            nc.sync.dma_start(
                out=wstage, in_=wdram.ap().rearrange("(ct p) m -> p ct m", p=128))
            nc.vector.tensor_copy(wbf, wstage)
        wod_bf = dpool.tile([512, C], BF, tag="wod_bf", name="wod_bf")
        nc.gpsimd.dma_start(out=wod_bf, in_=wo_d.ap())
        wo_bf = persist.tile([128, NG, C], BF, tag="wo_bf")
        nc.sync.dma_start(
            out=wo_bf, in_=wod_bf.rearrange("(g p) c -> p g c", p=128))

        for rnd in range(1, 4):
            nc.gpsimd.dma_start(
                out=xbf[rnd * 512:(rnd + 1) * 512, :],
                in_=x_d.ap()[rnd * 512:(rnd + 1) * 512, :],
            )
        # quarter 0 of xbf is unused now (round 0 transposed on-chip)

        # ones column of V
        ones_f32 = persist.tile([128, NTT, HC], F32, tag="ones")
        nc.vector.memset(ones_f32, 1.0)
        nc.vector.tensor_copy(V[:, :, :, 64], ones_f32)

        for rnd in range(4):
            q0 = rnd * 512  # first token of this quarter

            # ---- xT quarter via hardware DMA-transpose ----
            if rnd == 0:
                xTq = xTq0
            else:
                xTq = [work.tile([128, 512], BF, tag=f"xTq{ct}",
                                 name=f"xTq{ct}", bufs=2)
                       for ct in range(NCT)]
                for ct in range(NCT):
                    nc.sync.dma_start_transpose(
                        out=xTq[ct],
                        in_=xbf[q0:q0 + 512, ct * 128:(ct + 1) * 128]
                    )

            # ---- qT/kT for this quarter ----
            qTq = []
            for g in range(NG):
                pqk = ps.tile([128, 1024], F32, tag="pp", name="pqk")
                for ct in range(NCT):
                    nc.tensor.matmul(
                        pqk[:, 0:512],
                        wq_bf[:, ct, g * 128:(g + 1) * 128],
                        xTq[ct],
                        start=(ct == 0), stop=(ct == NCT - 1),
                    )
                    nc.tensor.matmul(
                        pqk[:, 512:1024],
                        wk_bf[:, ct, g * 128:(g + 1) * 128],
                        xTq[ct],
                        start=(ct == 0), stop=(ct == NCT - 1),
                    )
                qq = work.tile([128, 512], BF, tag=f"qTq{g}", bufs=2,
                               name=f"qTq{g}")
                nc.vector.tensor_copy(qq, pqk[:, 0:512])
                qTq.append(qq)
                nc.vector.tensor_copy(kT[g][:, q0:q0 + 512], pqk[:, 512:1024])

            # ---- V for this quarter (two tt-pairs per psum tile) ----
            for half in range(2):
                pv = ps.tile([128, 1024], F32, tag="pp", name="pv")
                for ct in range(NCT):
                    for sub in range(2):
                        jl = half * 2 + sub
                        nc.tensor.matmul(
                            pv[:, sub * 512:(sub + 1) * 512],
                            xTq[ct][:, jl * 128:(jl + 1) * 128],
                            wv_bf[:, ct, :],
                            start=(ct == 0), stop=(ct == NCT - 1),
                        )
                for sub in range(2):
                    tt = rnd * 4 + half * 2 + sub
                    for h in range(HC):
                        nc.vector.tensor_copy(
                            V[:, tt, h, 0:64],
                            pv[:, sub * 512 + h * 64: sub * 512 + h * 64 + 64],
                        )

            # ---- attention: q-block rnd for every group ----
            # Heads sequential, 2-kt score batches: 2-matmul bursts into a
            # [128,1024] psum span, one exp, causal select on diagonal
            # blocks, then a 2-matmul AV burst.
            qb = rnd
            nkt = 4 * (qb + 1)
            attTq = []
            for g in range(NG):
                att = work.tile([128, 512], BF, tag=f"attTq{g}", bufs=2,
                                name=f"attTq{g}")
                for hh in range(2):
                    head = 2 * g + hh
                    r0, r1 = 64 * hh, 64 * hh + 64
                    tp = (64 * hh, 0)
                    av = ps.tile([65, 512], F32, tag=f"av{hh}", name="av")
                    for b0 in range(0, nkt, 2):
                        sc = ps.tile([128, 1024], F32, tag="sc", bufs=2, name="sc")
                        for m in range(2):
                            nc.tensor.matmul(
                                sc[:, m * 512:(m + 1) * 512],
                                kT[g][r0:r1, (b0 + m) * 128:(b0 + m + 1) * 128],
                                qTq[g][r0:r1, :],
                                start=True, stop=True,
                                tile_position=tp,
                            )
                        wT = work.tile([128, 1024], BF, tag="wT", bufs=3)
                        nc.scalar.activation(wT, sc, EXP, scale=SCALE)
                        for m in range(2):
                            j = b0 + m - 4 * qb
                            if j >= 0:  # diagonal 128-block: causal select
                                ncols = 128 * j + 128
                                nc.gpsimd.affine_select(
                                    out=wT[:, m * 512:m * 512 + ncols],
                                    in_=wT[:, m * 512:m * 512 + ncols],
                                    compare_op=mybir.AluOpType.is_ge,
                                    fill=0.0,
                                    base=-128 * j,
                                    pattern=[[1, ncols]],
                                    channel_multiplier=-1,
                                )
                        for m in range(2):
                            kt = b0 + m
                            nc.tensor.matmul(
                                av, V[:, kt, head, :],
                                wT[:, m * 512:(m + 1) * 512],
                                start=(kt == 0), stop=(kt == nkt - 1),
                            )
                    # stage off PSUM, normalize off the critical path
                    avc = work.tile([65, 512], F32, tag="avc", bufs=4, name="avc")
                    nc.vector.tensor_copy(avc, av)
                    rec = work.tile([65, 512], F32, tag="rec", bufs=4, name="rec")
                    nc.vector.reciprocal(rec[64:65, :], avc[64:65, :])
                    rec_d = dpool.tile([1, 512], F32, tag="rec_d", bufs=4,
                                       name="rec_d")
                    nc.sync.dma_start(out=rec_d, in_=rec[64:65, :])
                    rep = work.tile([64, 512], F32, tag="rep", bufs=4, name="rep")
                    nc.sync.dma_start(
                        out=rep,
                        in_=bass.AP(rec_d.tensor, rec_d.offset,
                                    [[0, 64], [1, 512]]),
                    )
                    if hh == 0:
                        nc.vector.tensor_mul(att[0:64, :], avc[0:64, :], rep)
                    else:
                        tmpB = work.tile([64, 512], BF, tag="tmpB", bufs=2,
                                         name="tmpB")
                        nc.vector.tensor_mul(tmpB, avc[0:64, :], rep)
                        nc.sync.dma_start(out=att[64:128, :], in_=tmpB)
                attTq.append(att)

            # ---- out projection for this quarter's q rows ----
            for qtl in range(4):
                qt = rnd * 4 + qtl
                psy = ps.tile([128, 1024], F32, tag="pp", name="psy")
                for g in range(NG):
                    for half in range(2):
                        nc.tensor.matmul(
                            psy[:, half * 512:(half + 1) * 512],
                            attTq[g][:, qtl * 128:(qtl + 1) * 128],
                            wo_bf[:, g, half * 512:(half + 1) * 512],
                            start=(g == 0),
                            stop=(g == NG - 1),
                        )
                y_sb = work.tile([128, C], F32, tag="y_sb", bufs=2, name="y_sb")
                nc.vector.tensor_copy(y_sb, psy)
                nc.sync.dma_start(
                    out=y_d.ap()[qt * 128:(qt + 1) * 128, :], in_=y_sb
                )

    nc.compile()
    return nc


_NC_CACHE = None


def _get_nc():
    global _NC_CACHE
    if _NC_CACHE is None:
        _NC_CACHE = build_nc()
    return _NC_CACHE


def kernel(x, w_qkv, w_out, _trace=False):
    B = x.shape[0]
    x = np.ascontiguousarray(x, dtype=np.float32)
    w_qkv = np.ascontiguousarray(w_qkv, dtype=np.float32)
    w_out = np.ascontiguousarray(w_out, dtype=np.float32)

    nc = _get_nc()
    in_maps = []
    for core in range(8):
        b = core % B
        hbase = (core // B) * HC
        lo, hi = hbase * D, hbase * D + HC * D
        in_maps.append({
            "x": x[b],
            "wq": np.ascontiguousarray(w_qkv[:, lo:hi]),
            "wk": np.ascontiguousarray(w_qkv[:, C + lo:C + hi]),
            "wv": np.ascontiguousarray(w_qkv[:, 2 * C + lo:2 * C + hi]),
            "wo": np.ascontiguousarray(w_out[lo:hi, :]),
        })

    res = run_bass_kernel_spmd(nc, in_maps, core_ids=list(range(8)), trace=_trace)
    ys = [r["y"] for r in res.results]
    out = np.empty((B, T, C), dtype=np.float32)
    for b in range(B):
        out[b] = ys[b] + ys[b + B]
    if _trace:
        return out, res
    return out



# revision 2
# speedup vs baseline: 1.3918x; 1.3918x over previous
"""Causal self-attention for trn2, 8 NeuronCores.

Problem: x[4,2048,1024] @ w_qkv[1024,3072] -> causal MHA (16 heads, d=64)
-> @ w_out[1024,1024].

Sharding: core c handles batch b=c%4 and heads hbase=8*(c//4)..hbase+8
(data parallel on B x tensor parallel on heads). Each core computes the
partial out-projection y_c = att_slice @ w_out[slice]; the host sums the
two partials per batch.

v5 (from v4 trace analysis: 188us PE idle, HAM cold 320us, RECIPROCAL
107us on DVE):
- The two heads of a group run their score matmuls back-to-back at PE
  row-groups (0,0)/(64,0) so the K=64 matmuls execute concurrently.
  Both heads share one [128,1024] f32 PSUM score tile (1 k-tile each)
  and one exp covers both heads' scores.
- Diagonal k-tiles trim N to the causally-needed columns; the causal
  select shrinks to the [128]-wide triangular band.
- Softmax reciprocals are batched per head-pair as a [128,8] DVE op via
  a DRAM gather (replaces per-head [1,512] single-partition reciprocals).
- Projection/out-projection matmuls are emitted in chunks interleaved
  between attention steps (filler queue) so the PE queue always has
  ready work while ScalarE paces the exps; keeps HAM at full clock.
- Attention inner loop is software-pipelined: scores(kt+1) issue before
  AV(kt) so the exp latency is hidden.
"""

import sys

for p in ("/opt/trn_rl_repo", "/opt/pypackages"):
    if p not in sys.path:
        sys.path.insert(0, p)

import contextlib
from collections import deque

import numpy as np

import concourse.bass as bass
import concourse.mybir as mybir
import concourse.tile as tile
from concourse import bacc
from concourse.bass_utils import run_bass_kernel_spmd
from concourse.masks import make_identity

F32 = mybir.dt.float32
BF = mybir.dt.bfloat16
EXP = mybir.ActivationFunctionType.Exp

T = 2048          # sequence length
C = 1024          # model dim
HC = 8            # heads per core
D = 64            # head dim
NG = 4            # head-groups of 2 per core
NCT = C // 128    # 8 contraction tiles
NTT = T // 128    # 16 token tiles
SCALE = 0.125     # 1/sqrt(D)


def build_nc():
    nc = bacc.Bacc("TRN2", target_bir_lowering=False, debug=False)

    x_d = nc.dram_tensor("x", [T, C], F32, kind="ExternalInput")
    wq_d = nc.dram_tensor("wq", [C, 512], F32, kind="ExternalInput")
    wk_d = nc.dram_tensor("wk", [C, 512], F32, kind="ExternalInput")
    wv_d = nc.dram_tensor("wv", [C, 512], F32, kind="ExternalInput")
    wo_d = nc.dram_tensor("wo", [512, C], F32, kind="ExternalInput")
    y_d = nc.dram_tensor("y", [T, C], F32, kind="ExternalOutput")

    with tile.TileContext(nc) as tc, contextlib.ExitStack() as ctx:
        persist = ctx.enter_context(tc.tile_pool(name="persist", bufs=1))
        work = ctx.enter_context(tc.tile_pool(name="work", bufs=1))
        ps = ctx.enter_context(tc.tile_pool(name="ps", bufs=1, space="PSUM"))
        dpool = ctx.enter_context(tc.tile_pool(name="dram", bufs=1, space="DRAM"))

        kT = [persist.tile([128, T], BF, tag=f"kT{g}", name=f"kT{g}")
              for g in range(NG)]
        V = persist.tile([128, NTT, HC, 65], BF, tag="V")

        # x -> bf16 DRAM scratch for rounds 1-3 (contiguous SWDGE cast-DMA;
        # strided cast-DMAs truncate instead of rounding).
        xbf = dpool.tile([T, C], BF, tag="xbf", name="xbf")
        for rnd in range(1, 4):
            nc.gpsimd.dma_start(
                out=xbf[rnd * 512:(rnd + 1) * 512, :],
                in_=x_d.ap()[rnd * 512:(rnd + 1) * 512, :],
            )
        wod_bf = dpool.tile([512, C], BF, tag="wod_bf", name="wod_bf")
        nc.gpsimd.dma_start(out=wod_bf, in_=wo_d.ap())

        # round 0's xT via on-chip PE transposes so TensorE starts early
        ident = persist.tile([128, 128], F32, tag="ident", name="ident")
        make_identity(nc, ident)
        xTq0 = work.tile([128, NCT, 512], BF, tag="xTq", bufs=2, name="xTq0")
        for j in range(4):
            x_nat = work.tile([128, C], F32, tag="x_nat", bufs=2, name="x_nat")
            nc.sync.dma_start(out=x_nat, in_=x_d.ap()[j * 128:(j + 1) * 128, :])
            tp0 = ps.tile([128, NCT, 128], F32, tag="sc", bufs=2, name="tp0")
            for ct in range(NCT):
                nc.tensor.transpose(tp0[:, ct, :],
                                    x_nat[:, ct * 128:(ct + 1) * 128], ident)
            nc.vector.tensor_copy(xTq0[:, :, j * 128:(j + 1) * 128], tp0)

        # qkv weights: direct f32 loads + DVE casts (round-0 projection must
        # not wait on the SWDGE chain); wo keeps the DRAM-bounce cast.
        wq_bf = persist.tile([128, NCT, 512], BF, tag="wq_bf")
        wk_bf = persist.tile([128, NCT, 512], BF, tag="wk_bf")
        wv_bf = persist.tile([128, NCT, 512], BF, tag="wv_bf")
        for wdram, wbf in ((wq_d, wq_bf), (wk_d, wk_bf), (wv_d, wv_bf)):
            wstage = work.tile([128, NCT, 512], F32, tag="wstage", bufs=2,
                               name="wstage")
            nc.sync.dma_start(
                out=wstage, in_=wdram.ap().rearrange("(ct p) m -> p ct m", p=128))
            nc.vector.tensor_copy(wbf, wstage)
        wo_bf = persist.tile([128, NG, C], BF, tag="wo_bf")
        nc.sync.dma_start(
            out=wo_bf, in_=wod_bf.rearrange("(g p) c -> p g c", p=128))

        # ones column of V (AV matmul row 64 = softmax denominator)
        ones_f32 = persist.tile([128, NTT, HC], F32, tag="ones")
        nc.vector.memset(ones_f32, 1.0)
        nc.vector.tensor_copy(V[:, :, :, 64], ones_f32)

        # ---- filler queue: deferred PE work interleaved into attention ----
        fillq = deque()

        def pump(n):
            k = 0
            while fillq and k < n:
                fillq.popleft()()
                k += 1

        def push_qk_proj(r, xTq_t, qq_tiles):
            for g in range(NG):
                pqk = ps.tile([128, 1024], F32, tag="pp", name=f"pqk{r}{g}")

                def chunk(part, g=g, pqk=pqk, qq=qq_tiles[g]):
                    wbf = wq_bf if part < 2 else wk_bf
                    osl = slice(0, 512) if part < 2 else slice(512, 1024)
                    cts = range(0, 4) if part % 2 == 0 else range(4, 8)
                    for ct in cts:
                        nc.tensor.matmul(
                            pqk[:, osl],
                            wbf[:, ct, g * 128:(g + 1) * 128],
                            xTq_t[:, ct, :],
                            start=(ct == 0), stop=(ct == NCT - 1),
                        )
                    if part == 1:
                        nc.vector.tensor_copy(qq, pqk[:, 0:512])
                    elif part == 3:
                        nc.vector.tensor_copy(
                            kT[g][:, r * 512:(r + 1) * 512], pqk[:, 512:1024])

                for part in range(4):
                    fillq.append(lambda part=part, c=chunk: c(part))

        def push_v_proj(r, xTq_t):
            for half in range(2):
                pv = ps.tile([128, 2, HC, 64], F32, tag="pp",
                             name=f"pv{r}{half}")

                def chunk(part, half=half, pv=pv):
                    for ct in (2 * part, 2 * part + 1):
                        for sub in range(2):
                            jl = half * 2 + sub
                            nc.tensor.matmul(
                                pv[:, sub],
                                xTq_t[:, ct, jl * 128:(jl + 1) * 128],
                                wv_bf[:, ct, :],
                                start=(ct == 0), stop=(ct == NCT - 1),
                            )
                    if part == 3:
                        for sub in range(2):
                            tt = r * 4 + half * 2 + sub
                            nc.vector.tensor_copy(V[:, tt, :, 0:64], pv[:, sub])

                for part in range(4):
                    fillq.append(lambda part=part, c=chunk: c(part))

        def push_normalize(r, state, rc_dr):
            att_tiles, avcs = state

            def chunk():
                for g in range(NG):
                    rc_sb = work.tile([128, 8], F32, tag="rc_sb", bufs=4,
                                      name="rc_sb")
                    nc.vector.reciprocal(rc_sb, avcs[g][2])
                    nc.sync.dma_start(
                        out=bass.AP(rc_dr.tensor, rc_dr.offset + g * 1024,
                                    [[8, 128], [1, 8]]),
                        in_=rc_sb,
                    )
                for g in range(NG):
                    for hh in range(2):
                        rep = work.tile([64, 512], F32, tag="rep", bufs=4,
                                        name="rep")
                        nc.sync.dma_start(
                            out=rep,
                            in_=bass.AP(rc_dr.tensor,
                                        rc_dr.offset + (2 * g + hh) * 512,
                                        [[0, 64], [1, 512]]),
                        )
                        if hh == 0:
                            nc.vector.tensor_mul(
                                att_tiles[g][0:64, :], avcs[g][0][0:64, :], rep)
                        else:
                            tmpB = work.tile([64, 512], BF, tag="tmpB", bufs=2,
                                             name="tmpB")
                            nc.vector.tensor_mul(tmpB, avcs[g][1][0:64, :], rep)
                            nc.sync.dma_start(
                                out=att_tiles[g][64:128, :], in_=tmpB)

            fillq.append(chunk)

        def push_outproj(r, att_tiles):
            for qtl in range(4):
                psy = ps.tile([128, 1024], F32, tag="pp", name=f"psy{r}{qtl}")

                def chunk(part, qtl=qtl, psy=psy):
                    for g in (2 * part, 2 * part + 1):
                        for hf in range(2):
                            nc.tensor.matmul(
                                psy[:, hf * 512:(hf + 1) * 512],
                                att_tiles[g][:, qtl * 128:(qtl + 1) * 128],
                                wo_bf[:, g, hf * 512:(hf + 1) * 512],
                                start=(g == 0), stop=(g == NG - 1),
                            )
                    if part == 1:
                        qt = r * 4 + qtl
                        y_sb = work.tile([128, C], F32, tag="y_sb", bufs=2,
                                         name="y_sb")
                        nc.vector.tensor_copy(y_sb, psy)
                        nc.sync.dma_start(
                            out=y_d.ap()[qt * 128:(qt + 1) * 128, :], in_=y_sb)

                for part in range(2):
                    fillq.append(lambda part=part, c=chunk: c(part))

        def attention_round(r, qq_tiles, dn_dr, pump_rate, start_pump):
            qb = r
            nkt = 4 * (qb + 1)
            att_tiles = []
            avcs = []
            step = 0
            for g in range(NG):
                h0, h1 = 2 * g, 2 * g + 1
                avA = ps.tile([65, 512], F32, tag="avA", name=f"avA{r}{g}")
                avB = ps.tile([65, 512], F32, tag="avB", name=f"avB{r}{g}")
                att = work.tile([128, 512], BF, tag=f"att{g}", bufs=2,
                                name=f"att{g}")
                qq = qq_tiles[g]
                pend = None
                for kt in range(nkt + 1):
                    if kt < nkt:
                        j = kt - 4 * qb
                        n0 = 128 * j if j > 0 else 0
                        sc = ps.tile([128, 1024], F32, tag="sc", bufs=2,
                                     name="sc")
                        nc.tensor.matmul(
                            sc[:, n0:512],
                            kT[g][0:64, kt * 128:(kt + 1) * 128],
                            qq[0:64, n0:512],
                            start=True, stop=True, tile_position=(0, 0),
                        )
                        nc.tensor.matmul(
                            sc[:, 512 + n0:1024],
                            kT[g][64:128, kt * 128:(kt + 1) * 128],
                            qq[64:128, n0:512],
                            start=True, stop=True, tile_position=(64, 0),
                        )
                        wT = work.tile([128, 1024], BF, tag="wT", bufs=3,
                                       name="wT")
                        if n0 > 0:
                            nc.scalar.activation(wT[:, n0:512], sc[:, n0:512],
                                                 EXP, scale=SCALE)
                            nc.scalar.activation(wT[:, 512 + n0:1024],
                                                 sc[:, 512 + n0:1024],
                                                 EXP, scale=SCALE)
                        else:
                            nc.scalar.activation(wT, sc, EXP, scale=SCALE)
                        if j >= 0:  # diagonal block: triangular causal select
                            for base_col in (n0, 512 + n0):
                                nc.gpsimd.affine_select(
                                    out=wT[:, base_col:base_col + 128],
                                    in_=wT[:, base_col:base_col + 128],
                                    compare_op=mybir.AluOpType.is_ge,
                                    fill=0.0, base=0,
                                    pattern=[[1, 128]],
                                    channel_multiplier=-1,
                                )
                        cur = (wT, kt, n0)
                    if pend is not None:
                        wTp, ktp, n0p = pend
                        nc.tensor.matmul(
                            avA[:, n0p:512], V[:, ktp, h0, :],
                            wTp[:, n0p:512],
                            start=(ktp == 0), stop=(ktp == nkt - 1),
                        )
                        nc.tensor.matmul(
                            avB[:, n0p:512], V[:, ktp, h1, :],
                            wTp[:, 512 + n0p:1024],
                            start=(ktp == 0), stop=(ktp == nkt - 1),
                        )
                        if step >= start_pump:
                            pump(pump_rate)
                        step += 1
                    if kt < nkt:
                        pend = cur
                # drain AV + stage denominators to DRAM for batched recip
                avcA = work.tile([65, 512], F32, tag="avcA", bufs=4,
                                 name="avcA")
                avcB = work.tile([65, 512], F32, tag="avcB", bufs=4,
                                 name="avcB")
                nc.vector.tensor_copy(avcA, avA)
                nc.vector.tensor_copy(avcB, avB)
                nc.sync.dma_start(out=dn_dr[2 * g:2 * g + 1, :],
                                  in_=avcA[64:65, :])
                nc.sync.dma_start(out=dn_dr[2 * g + 1:2 * g + 2, :],
                                  in_=avcB[64:65, :])
                dn_sb = work.tile([128, 8], F32, tag="dn_sb", bufs=4,
                                  name="dn_sb")
                nc.sync.dma_start(
                    out=dn_sb,
                    in_=bass.AP(dn_dr.tensor, dn_dr.offset + g * 1024,
                                [[8, 128], [1, 8]]),
                )
                att_tiles.append(att)
                avcs.append((avcA, avcB, dn_sb))
            return att_tiles, avcs

        # ---- main pipeline over T-quarters ----
        qq_tiles = {0: [work.tile([128, 512], BF, tag=f"qq{g}", bufs=2,
                                  name=f"qq{g}") for g in range(NG)]}
        push_qk_proj(0, xTq0, qq_tiles[0])
        push_v_proj(0, xTq0)
        pump(len(fillq))  # round-0 projections emitted inline

        xTq_t = {0: xTq0}
        states = {}
        rc_ds = {}
        pump_rates = {0: 2, 1: 1, 2: 1, 3: 1}
        start_pumps = {0: 0, 1: 0, 2: 0, 3: 8}
        for r in range(4):
            # hardware DMA-transpose for round r+1's xT (cast already done)
            if r + 1 <= 3:
                t = work.tile([128, NCT, 512], BF, tag="xTq", bufs=2,
                              name=f"xTq{r + 1}")
                xTq_t[r + 1] = t
                q0 = (r + 1) * 512
                for ct in range(NCT):
                    nc.sync.dma_start_transpose(
                        out=t[:, ct, :],
                        in_=xbf[q0:q0 + 512, ct * 128:(ct + 1) * 128])
            dn_dr = dpool.tile([8, 512], F32, tag="dn_d", bufs=2,
                               name=f"dn_d{r}")
            rc_ds[r] = dpool.tile([8, 512], F32, tag="rc_d", bufs=2,
                                  name=f"rc_d{r}")
            if r < 3:
                qq_tiles[r + 1] = [work.tile([128, 512], BF, tag=f"qq{g}",
                                             bufs=2, name=f"qq{g}")
                                   for g in range(NG)]
                push_qk_proj(r + 1, xTq_t[r + 1], qq_tiles[r + 1])
                push_v_proj(r + 1, xTq_t[r + 1])
            if r > 0:
                push_normalize(r - 1, states[r - 1], rc_ds[r - 1])
                push_outproj(r - 1, states[r - 1][0])
            states[r] = attention_round(r, qq_tiles[r], dn_dr,
                                        pump_rates[r], start_pumps[r])
            pump(len(fillq))  # drain leftovers at round boundary
        push_normalize(3, states[3], rc_ds[3])
        push_outproj(3, states[3][0])
        pump(len(fillq))

    nc.compile()
    return nc


_NC_CACHE = None


def _get_nc():
    global _NC_CACHE
    if _NC_CACHE is None:
        _NC_CACHE = build_nc()
    return _NC_CACHE


def kernel(x, w_qkv, w_out, _trace=False):
    B = x.shape[0]
    x = np.ascontiguousarray(x, dtype=np.float32)
    w_qkv = np.ascontiguousarray(w_qkv, dtype=np.float32)
    w_out = np.ascontiguousarray(w_out, dtype=np.float32)

    nc = _get_nc()
    in_maps = []
    for core in range(8):
        b = core % B
        hbase = (core // B) * HC
        lo, hi = hbase * D, hbase * D + HC * D
        in_maps.append({
            "x": x[b],
            "wq": np.ascontiguousarray(w_qkv[:, lo:hi]),
            "wk": np.ascontiguousarray(w_qkv[:, C + lo:C + hi]),
            "wv": np.ascontiguousarray(w_qkv[:, 2 * C + lo:2 * C + hi]),
            "wo": np.ascontiguousarray(w_out[lo:hi, :]),
        })

    res = run_bass_kernel_spmd(nc, in_maps, core_ids=list(range(8)), trace=_trace)
    ys = [r["y"] for r in res.results]
    out = np.empty((B, T, C), dtype=np.float32)
    for b in range(B):
        out[b] = ys[b] + ys[b + B]
    if _trace:
        return out, res
    return out


# revision 10
# speedup vs baseline: 1.4802x; 1.0635x over previous
"""Causal self-attention for trn2, 8 NeuronCores.

Problem: x[4,2048,1024] @ w_qkv[1024,3072] -> causal MHA (16 heads, d=64)
-> @ w_out[1024,1024].

Sharding: core c handles batch b=c%4 and heads hbase=8*(c//4)..hbase+8
(data parallel on B x tensor parallel on heads). Each core computes the
partial out-projection y_c = att_slice @ w_out[slice]; the host sums the
two partials per batch.

v5 (from v4 trace analysis: 188us PE idle, HAM cold 320us, RECIPROCAL
107us on DVE):
- The two heads of a group run their score matmuls back-to-back at PE
  row-groups (0,0)/(64,0) so the K=64 matmuls execute concurrently.
  Both heads share one [128,1024] f32 PSUM score tile (1 k-tile each)
  and one exp covers both heads' scores.
- Diagonal k-tiles trim N to the causally-needed columns; the causal
  select shrinks to the [128]-wide triangular band.
- Softmax reciprocals are batched per head-pair as a [128,8] DVE op via
  a DRAM gather (replaces per-head [1,512] single-partition reciprocals).
- Projection/out-projection matmuls are emitted in chunks interleaved
  between attention steps (filler queue) so the PE queue always has
  ready work while ScalarE paces the exps; keeps HAM at full clock.
- Attention inner loop is software-pipelined: scores(kt+1) issue before
  AV(kt) so the exp latency is hidden.
"""

import sys

for p in ("/opt/trn_rl_repo", "/opt/pypackages"):
    if p not in sys.path:
        sys.path.insert(0, p)

import contextlib
from collections import deque

import numpy as np

import concourse.bass as bass
import concourse.mybir as mybir
import concourse.tile as tile
from concourse import bacc
from concourse.bass_utils import run_bass_kernel_spmd
from concourse.masks import make_identity

F32 = mybir.dt.float32
BF = mybir.dt.bfloat16
EXP = mybir.ActivationFunctionType.Exp

T = 2048          # sequence length
C = 1024          # model dim
HC = 8            # heads per core
D = 64            # head dim
NG = 4            # head-groups of 2 per core
NCT = C // 128    # 8 contraction tiles
NTT = T // 128    # 16 token tiles
SCALE = 0.125     # 1/sqrt(D)


def build_nc():
    nc = bacc.Bacc("TRN2", target_bir_lowering=False, debug=False)

    x_d = nc.dram_tensor("x", [T, C], F32, kind="ExternalInput")
    wq_d = nc.dram_tensor("wq", [C, 512], F32, kind="ExternalInput")
    wk_d = nc.dram_tensor("wk", [C, 512], F32, kind="ExternalInput")
    wv_d = nc.dram_tensor("wv", [C, 512], F32, kind="ExternalInput")
    wo_d = nc.dram_tensor("wo", [512, C], F32, kind="ExternalInput")
    y_d = nc.dram_tensor("y", [T, C], F32, kind="ExternalOutput")

    with tile.TileContext(nc) as tc, contextlib.ExitStack() as ctx:
        persist = ctx.enter_context(tc.tile_pool(name="persist", bufs=1))
        work = ctx.enter_context(tc.tile_pool(name="work", bufs=1))
        ps = ctx.enter_context(tc.tile_pool(name="ps", bufs=1, space="PSUM"))
        dpool = ctx.enter_context(tc.tile_pool(name="dram", bufs=1, space="DRAM"))

        kT = [persist.tile([128, T], BF, tag=f"kT{g}", name=f"kT{g}")
              for g in range(NG)]
        V = persist.tile([128, NTT, HC, 65], BF, tag="V")

        # Rounds 0 AND 1 get xT via on-chip PE transposes (PE is idle in the
        # prologue anyway and this avoids waiting on the SWDGE cast-DMA
        # chain); rounds 2-3 use the bf16 DRAM scratch + HW DMA-transpose.
        ident = persist.tile([128, 128], F32, tag="ident", name="ident")
        make_identity(nc, ident)
        xTq0 = work.tile([128, NCT, 512], BF, tag="xTq", bufs=3, name="xTq0")
        xTq1 = work.tile([128, NCT, 512], BF, tag="xTq", bufs=3, name="xTq1")
        wq_bf = persist.tile([128, NCT, 512], BF, tag="wq_bf")
        wk_bf = persist.tile([128, NCT, 512], BF, tag="wk_bf")
        wv_bf = persist.tile([128, NCT, 512], BF, tag="wv_bf")

        x_nats = []
        for j in range(8):
            x_nat = work.tile([128, C], F32, tag="x_nat", bufs=4, name="x_nat")
            nc.sync.dma_start(out=x_nat, in_=x_d.ap()[j * 128:(j + 1) * 128, :])
            x_nats.append(x_nat)
        wstages = []
        for wdram in (wq_d, wk_d, wv_d):
            wstage = work.tile([128, NCT, 512], F32, tag="wstage", bufs=2,
                               name="wstage")
            nc.sync.dma_start(
                out=wstage, in_=wdram.ap().rearrange("(ct p) m -> p ct m", p=128))
            wstages.append(wstage)
        wcasts = {3: 0, 5: 1, 7: 2}
        for j in range(8):
            xTq_dst = xTq0 if j < 4 else xTq1
            jj = j % 4
            tp0 = ps.tile([128, NCT, 128], F32, tag="sc", bufs=2, name="tp0")
            for ct in range(NCT):
                nc.tensor.transpose(tp0[:, ct, :],
                                    x_nats[j][:, ct * 128:(ct + 1) * 128], ident)
            nc.vector.tensor_copy(xTq_dst[:, :, jj * 128:(jj + 1) * 128], tp0)
            if j in wcasts:
                i = wcasts[j]
                nc.vector.tensor_copy((wq_bf, wk_bf, wv_bf)[i], wstages[i])

        # x -> bf16 DRAM scratch for rounds 2-3 (contiguous SWDGE cast-DMA;
        # strided cast-DMAs truncate instead of rounding). Emitted after the
        # f32 loads above so they don't hog the DMA queues early.
        xbf = dpool.tile([T, C], BF, tag="xbf", name="xbf")
        for rnd in range(2, 4):
            nc.gpsimd.dma_start(
                out=xbf[rnd * 512:(rnd + 1) * 512, :],
                in_=x_d.ap()[rnd * 512:(rnd + 1) * 512, :],
            )
        wod_bf = dpool.tile([512, C], BF, tag="wod_bf", name="wod_bf")
        nc.gpsimd.dma_start(out=wod_bf, in_=wo_d.ap())
        wo_bf = persist.tile([128, NG, C], BF, tag="wo_bf")
        nc.sync.dma_start(
            out=wo_bf, in_=wod_bf.rearrange("(g p) c -> p g c", p=128))

        # ones column of V (AV matmul row 64 = softmax denominator)
        ones_f32 = persist.tile([128, NTT, HC], F32, tag="ones")
        nc.vector.memset(ones_f32, 1.0)
        nc.vector.tensor_copy(V[:, :, :, 64], ones_f32)

        # ---- filler queue: deferred PE work interleaved into attention ----
        fillq = deque()

        def pump(n):
            k = 0
            while fillq and k < n:
                fillq.popleft()()
                k += 1

        def push_qk_proj(r, xTq_t, qq_tiles):
            for g in range(NG):
                pqk = ps.tile([128, 1024], F32, tag="pp", name=f"pqk{r}{g}")

                def chunk(part, g=g, pqk=pqk, qq=qq_tiles[g]):
                    wbf = wq_bf if part < 2 else wk_bf
                    osl = slice(0, 512) if part < 2 else slice(512, 1024)
                    cts = range(0, 4) if part % 2 == 0 else range(4, 8)
                    for ct in cts:
                        nc.tensor.matmul(
                            pqk[:, osl],
                            wbf[:, ct, g * 128:(g + 1) * 128],
                            xTq_t[:, ct, :],
                            start=(ct == 0), stop=(ct == NCT - 1),
                        )
                    if part == 1:
                        nc.vector.tensor_copy(qq, pqk[:, 0:512])
                    elif part == 3:
                        nc.vector.tensor_copy(
                            kT[g][:, r * 512:(r + 1) * 512], pqk[:, 512:1024])

                for part in range(4):
                    fillq.append(lambda part=part, c=chunk: c(part))

        def push_v_proj(r, xTq_t):
            for half in range(2):
                pv = ps.tile([128, 2, HC, 64], F32, tag="pp",
                             name=f"pv{r}{half}")

                def chunk(part, half=half, pv=pv):
                    for ct in (2 * part, 2 * part + 1):
                        for sub in range(2):
                            jl = half * 2 + sub
                            nc.tensor.matmul(
                                pv[:, sub],
                                xTq_t[:, ct, jl * 128:(jl + 1) * 128],
                                wv_bf[:, ct, :],
                                start=(ct == 0), stop=(ct == NCT - 1),
                            )
                    if part == 3:
                        for sub in range(2):
                            tt = r * 4 + half * 2 + sub
                            nc.vector.tensor_copy(V[:, tt, :, 0:64], pv[:, sub])

                for part in range(4):
                    fillq.append(lambda part=part, c=chunk: c(part))

        def normalize_g(g, att, avc, dn_sb, rc_dr):
            rc_sb = work.tile([128, 8], F32, tag="rc_sb", bufs=4,
                              name="rc_sb")
            nc.vector.reciprocal(rc_sb, dn_sb)
            nc.sync.dma_start(
                out=bass.AP(rc_dr.tensor, rc_dr.offset + g * 1024,
                            [[8, 128], [1, 8]]),
                in_=rc_sb,
            )
            for hh in range(2):
                rep = work.tile([64, 512], F32, tag="rep", bufs=4,
                                name="rep")
                nc.sync.dma_start(
                    out=rep,
                    in_=bass.AP(rc_dr.tensor,
                                rc_dr.offset + (2 * g + hh) * 512,
                                [[0, 64], [1, 512]]),
                )
                if hh == 0:
                    nc.vector.tensor_mul(att[0:64, :], avc[0:64, 0:512], rep)
                else:
                    tmpB = work.tile([64, 512], BF, tag="tmpB", bufs=2,
                                     name="tmpB")
                    nc.vector.tensor_mul(tmpB, avc[0:64, 512:1024], rep)
                    nc.sync.dma_start(out=att[64:128, :], in_=tmpB)

        def push_normalize(r, state, rc_dr):
            att_tiles, avcs = state
            for g in range(NG):
                fillq.append(lambda g=g: normalize_g(
                    g, att_tiles[g], avcs[g][0], avcs[g][1], rc_dr))

        def push_outproj(r, att_tiles):
            for qtl in range(4):
                psy = ps.tile([128, 1024], F32, tag="pp", name=f"psy{r}{qtl}")

                def chunk(part, qtl=qtl, psy=psy):
                    for g in (2 * part, 2 * part + 1):
                        for hf in range(2):
                            nc.tensor.matmul(
                                psy[:, hf * 512:(hf + 1) * 512],
                                att_tiles[g][:, qtl * 128:(qtl + 1) * 128],
                                wo_bf[:, g, hf * 512:(hf + 1) * 512],
                                start=(g == 0), stop=(g == NG - 1),
                            )
                    if part == 1:
                        qt = r * 4 + qtl
                        y_sb = work.tile([128, C], F32, tag="y_sb", bufs=2,
                                         name="y_sb")
                        nc.vector.tensor_copy(y_sb, psy)
                        nc.sync.dma_start(
                            out=y_d.ap()[qt * 128:(qt + 1) * 128, :], in_=y_sb)

                for part in range(2):
                    fillq.append(lambda part=part, c=chunk: c(part))

        def attention_round(r, qq_tiles, dn_dr, rc_dr, pump_rate, start_pump,
                            inline_norm):
            qb = r
            nkt = 4 * (qb + 1)
            att_tiles = []
            avcs = []
            step = 0
            for g in range(NG):
                h0, h1 = 2 * g, 2 * g + 1
                av = ps.tile([65, 1024], F32, tag="av", name=f"av{r}{g}")
                avA = av[:, 0:512]
                avB = av[:, 512:1024]
                att = work.tile([128, 512], BF, tag=f"att{g}", bufs=2,
                                name=f"att{g}")
                qq = qq_tiles[g]
                pend = None
                for kt in range(nkt + 1):
                    if kt < nkt:
                        j = kt - 4 * qb
                        n0 = 128 * j if j > 0 else 0
                        sc = ps.tile([128, 1024], F32, tag="sc", bufs=2,
                                     name="sc")
                        nc.tensor.matmul(
                            sc[:, n0:512],
                            kT[g][0:64, kt * 128:(kt + 1) * 128],
                            qq[0:64, n0:512],
                            start=True, stop=True, tile_position=(0, 0),
                        )
                        nc.tensor.matmul(
                            sc[:, 512 + n0:1024],
                            kT[g][64:128, kt * 128:(kt + 1) * 128],
                            qq[64:128, n0:512],
                            start=True, stop=True, tile_position=(64, 0),
                        )
                        wT = work.tile([128, 1024], BF, tag="wT", bufs=3,
                                       name="wT")
                        if n0 > 0:
                            nc.scalar.activation(wT[:, n0:512], sc[:, n0:512],
                                                 EXP, scale=SCALE)
                            nc.scalar.activation(wT[:, 512 + n0:1024],
                                                 sc[:, 512 + n0:1024],
                                                 EXP, scale=SCALE)
                        else:
                            nc.scalar.activation(wT, sc, EXP, scale=SCALE)
                        if j >= 0:  # diagonal block: triangular causal select
                            for base_col in (n0, 512 + n0):
                                nc.gpsimd.affine_select(
                                    out=wT[:, base_col:base_col + 128],
                                    in_=wT[:, base_col:base_col + 128],
                                    compare_op=mybir.AluOpType.is_ge,
                                    fill=0.0, base=0,
                                    pattern=[[1, 128]],
                                    channel_multiplier=-1,
                                )
                        cur = (wT, kt, n0)
                    if pend is not None:
                        wTp, ktp, n0p = pend
                        nc.tensor.matmul(
                            av[:, n0p:512], V[:, ktp, h0, :],
                            wTp[:, n0p:512],
                            start=(ktp == 0), stop=(ktp == nkt - 1),
                        )
                        nc.tensor.matmul(
                            av[:, 512 + n0p:1024], V[:, ktp, h1, :],
                            wTp[:, 512 + n0p:1024],
                            start=(ktp == 0), stop=(ktp == nkt - 1),
                        )
                        if step >= start_pump:
                            pump(pump_rate)
                        step += 1
                    if kt < nkt:
                        pend = cur
                # drain AV + stage denominators to DRAM for batched recip
                avc = work.tile([65, 1024], F32, tag="avc", bufs=4,
                                name="avc")
                nc.vector.tensor_copy(avc, av)
                nc.sync.dma_start(out=dn_dr[g:g + 1, :], in_=avc[64:65, :])
                dn_sb = work.tile([128, 8], F32, tag="dn_sb", bufs=4,
                                  name="dn_sb")
                nc.sync.dma_start(
                    out=dn_sb,
                    in_=bass.AP(dn_dr.tensor, dn_dr.offset + g * 1024,
                                [[8, 128], [1, 8]]),
                )
                att_tiles.append(att)
                avcs.append((avc, dn_sb))
                if inline_norm:
                    normalize_g(g, att, avc, dn_sb, rc_dr)
            return att_tiles, avcs

        # ---- main pipeline over T-quarters ----
        qq_tiles = {0: [work.tile([128, 512], BF, tag=f"qq{g}", bufs=2,
                                  name=f"qq{g}") for g in range(NG)]}
        push_qk_proj(0, xTq0, qq_tiles[0])
        push_v_proj(0, xTq0)
        pump(len(fillq))  # round-0 projections emitted inline

        xTq_t = {0: xTq0, 1: xTq1}
        states = {}
        rc_ds = {}
        pump_rates = {0: 2, 1: 2, 2: 1, 3: 1}
        start_pumps = {0: 0, 1: 0, 2: 0, 3: 6}
        for r in range(4):
            if r == 0:
                # hardware DMA-transpose for rounds 2-3's xT; they start as
                # soon as the SWDGE casts land and the xTq buffers free up,
                # long before the round-2/3 projections need them.
                for rr in (2, 3):
                    t = work.tile([128, NCT, 512], BF, tag="xTq", bufs=3,
                                  name=f"xTq{rr}")
                    xTq_t[rr] = t
                    q0 = rr * 512
                    for ct in range(NCT):
                        nc.sync.dma_start_transpose(
                            out=t[:, ct, :],
                            in_=xbf[q0:q0 + 512, ct * 128:(ct + 1) * 128])
            dn_dr = dpool.tile([4, 1024], F32, tag="dn_d", bufs=2,
                               name=f"dn_d{r}")
            rc_ds[r] = dpool.tile([4, 1024], F32, tag="rc_d", bufs=2,
                                  name=f"rc_d{r}")
            if r < 3:
                qq_tiles[r + 1] = [work.tile([128, 512], BF, tag=f"qq{g}",
                                             bufs=2, name=f"qq{g}")
                                   for g in range(NG)]
                push_qk_proj(r + 1, xTq_t[r + 1], qq_tiles[r + 1])
                push_v_proj(r + 1, xTq_t[r + 1])
            if r > 0:
                push_normalize(r - 1, states[r - 1], rc_ds[r - 1])
                push_outproj(r - 1, states[r - 1][0])
            states[r] = attention_round(r, qq_tiles[r], dn_dr, rc_ds[r],
                                        pump_rates[r], start_pumps[r],
                                        inline_norm=(r == 3))
            pump(len(fillq))  # drain leftovers at round boundary
        push_outproj(3, states[3][0])
        pump(len(fillq))

    nc.compile()
    return nc


_NC_CACHE = None


def _get_nc():
    global _NC_CACHE
    if _NC_CACHE is None:
        _NC_CACHE = build_nc()
    return _NC_CACHE


def kernel(x, w_qkv, w_out, _trace=False):
    B = x.shape[0]
    x = np.ascontiguousarray(x, dtype=np.float32)
    w_qkv = np.ascontiguousarray(w_qkv, dtype=np.float32)
    w_out = np.ascontiguousarray(w_out, dtype=np.float32)

    nc = _get_nc()
    in_maps = []
    for core in range(8):
        b = core % B
        hbase = (core // B) * HC
        lo, hi = hbase * D, hbase * D + HC * D
        in_maps.append({
            "x": x[b],
            "wq": np.ascontiguousarray(w_qkv[:, lo:hi]),
            "wk": np.ascontiguousarray(w_qkv[:, C + lo:C + hi]),
            "wv": np.ascontiguousarray(w_qkv[:, 2 * C + lo:2 * C + hi]),
            "wo": np.ascontiguousarray(w_out[lo:hi, :]),
        })

    res = run_bass_kernel_spmd(nc, in_maps, core_ids=list(range(8)), trace=_trace)
    ys = [r["y"] for r in res.results]
    out = np.empty((B, T, C), dtype=np.float32)
    for b in range(B):
        out[b] = ys[b] + ys[b + B]
    if _trace:
        return out, res
    return out


# revision 16
# speedup vs baseline: 1.6088x; 1.0869x over previous
"""Causal self-attention for trn2, 8 NeuronCores.

Problem: x[4,2048,1024] @ w_qkv[1024,3072] -> causal MHA (16 heads, d=64)
-> @ w_out[1024,1024].

Sharding: core c handles batch b=c%4 and heads hbase=8*(c//4)..hbase+8
(data parallel on B x tensor parallel on heads). Each core computes the
partial out-projection y_c = att_slice @ w_out[slice]; the host sums the
two partials per batch.

v5 (from v4 trace analysis: 188us PE idle, HAM cold 320us, RECIPROCAL
107us on DVE):
- The two heads of a group run their score matmuls back-to-back at PE
  row-groups (0,0)/(64,0) so the K=64 matmuls execute concurrently.
  Both heads share one [128,1024] f32 PSUM score tile (1 k-tile each)
  and one exp covers both heads' scores.
- Diagonal k-tiles trim N to the causally-needed columns; the causal
  select shrinks to the [128]-wide triangular band.
- Softmax reciprocals are batched per head-pair as a [128,8] DVE op via
  a DRAM gather (replaces per-head [1,512] single-partition reciprocals).
- Projection/out-projection matmuls are emitted in chunks interleaved
  between attention steps (filler queue) so the PE queue always has
  ready work while ScalarE paces the exps; keeps HAM at full clock.
- Attention inner loop is software-pipelined: scores(kt+1) issue before
  AV(kt) so the exp latency is hidden.
"""

import sys

for p in ("/opt/trn_rl_repo", "/opt/pypackages"):
    if p not in sys.path:
        sys.path.insert(0, p)

import contextlib
from collections import deque

import numpy as np

import concourse.bass as bass
import concourse.mybir as mybir
import concourse.tile as tile
from concourse import bacc
from concourse.bass_utils import run_bass_kernel_spmd
from concourse.masks import make_identity

F32 = mybir.dt.float32
BF = mybir.dt.bfloat16
EXP = mybir.ActivationFunctionType.Exp

T = 2048          # sequence length
C = 1024          # model dim
HC = 8            # heads per core
D = 64            # head dim
NG = 4            # head-groups of 2 per core
NCT = C // 128    # 8 contraction tiles
NTT = T // 128    # 16 token tiles
SCALE = 0.125     # 1/sqrt(D)


def build_nc():
    nc = bacc.Bacc("TRN2", target_bir_lowering=False, debug=False)

    # Inputs are pre-cast to bf16 on the host: halves the HBM upload and
    # removes every on-device cast/staging step from the prologue.
    x_d = nc.dram_tensor("x", [T, C], BF, kind="ExternalInput")
    wq_d = nc.dram_tensor("wq", [C, 512], BF, kind="ExternalInput")
    wk_d = nc.dram_tensor("wk", [C, 512], BF, kind="ExternalInput")
    wv_d = nc.dram_tensor("wv", [C, 512], BF, kind="ExternalInput")
    wo_d = nc.dram_tensor("wo", [512, C], BF, kind="ExternalInput")
    y_d = nc.dram_tensor("y", [T, C], F32, kind="ExternalOutput")

    with tile.TileContext(nc) as tc, contextlib.ExitStack() as ctx:
        persist = ctx.enter_context(tc.tile_pool(name="persist", bufs=1))
        work = ctx.enter_context(tc.tile_pool(name="work", bufs=1))
        ps = ctx.enter_context(tc.tile_pool(name="ps", bufs=1, space="PSUM"))
        dpool = ctx.enter_context(tc.tile_pool(name="dram", bufs=1, space="DRAM"))

        kT = [persist.tile([128, T], BF, tag=f"kT{g}", name=f"kT{g}")
              for g in range(NG)]
        V = persist.tile([128, NTT, HC, 65], BF, tag="V")

        # xT for every round comes straight off x via hardware DMA-transpose
        # (x is already bf16 in DRAM). Round 0's tiles + q/k weights first so
        # the first projection can start ~5us in.
        wq_bf = persist.tile([128, NCT, 512], BF, tag="wq_bf")
        wk_bf = persist.tile([128, NCT, 512], BF, tag="wk_bf")
        wv_bf = persist.tile([128, NCT, 512], BF, tag="wv_bf")
        wo_bf = persist.tile([128, NG, C], BF, tag="wo_bf")

        xTq_t = {}

        def emit_xT(r):
            t = work.tile([128, NCT, 512], BF, tag="xTq", bufs=4,
                          name=f"xTq{r}")
            xTq_t[r] = t
            q0 = r * 512
            for ct in range(NCT):
                nc.sync.dma_start_transpose(
                    out=t[:, ct, :],
                    in_=x_d.ap()[q0:q0 + 512, ct * 128:(ct + 1) * 128])

        emit_xT(0)
        nc.sync.dma_start(
            out=wq_bf, in_=wq_d.ap().rearrange("(ct p) m -> p ct m", p=128))
        nc.sync.dma_start(
            out=wk_bf, in_=wk_d.ap().rearrange("(ct p) m -> p ct m", p=128))
        nc.sync.dma_start(
            out=wv_bf, in_=wv_d.ap().rearrange("(ct p) m -> p ct m", p=128))
        emit_xT(1)
        nc.sync.dma_start(
            out=wo_bf, in_=wo_d.ap().rearrange("(g p) c -> p g c", p=128))
        emit_xT(2)
        emit_xT(3)

        # ones column of V (AV matmul row 64 = softmax denominator)
        ones_f32 = persist.tile([128, NTT, HC], F32, tag="ones")
        nc.vector.memset(ones_f32, 1.0)
        nc.vector.tensor_copy(V[:, :, :, 64], ones_f32)

        # ---- filler queue: deferred PE work interleaved into attention ----
        fillq = deque()

        def pump(n):
            k = 0
            while fillq and k < n:
                fillq.popleft()()
                k += 1

        def push_qk_proj(r, xTq_t, qq_tiles):
            for g in range(NG):
                pqk = ps.tile([128, 1024], F32, tag="pp", name=f"pqk{r}{g}")

                def chunk(part, g=g, pqk=pqk, qq=qq_tiles[g]):
                    wbf = wq_bf if part < 2 else wk_bf
                    osl = slice(0, 512) if part < 2 else slice(512, 1024)
                    cts = range(0, 4) if part % 2 == 0 else range(4, 8)
                    for ct in cts:
                        nc.tensor.matmul(
                            pqk[:, osl],
                            wbf[:, ct, g * 128:(g + 1) * 128],
                            xTq_t[:, ct, :],
                            start=(ct == 0), stop=(ct == NCT - 1),
                        )
                    if part == 1:
                        nc.vector.tensor_copy(qq, pqk[:, 0:512])
                    elif part == 3:
                        nc.vector.tensor_copy(
                            kT[g][:, r * 512:(r + 1) * 512], pqk[:, 512:1024])

                for part in range(4):
                    fillq.append(lambda part=part, c=chunk: c(part))

        def push_v_proj(r, xTq_t):
            for half in range(2):
                pv = ps.tile([128, 2, HC, 64], F32, tag="pp",
                             name=f"pv{r}{half}")

                def chunk(part, half=half, pv=pv):
                    for ct in (2 * part, 2 * part + 1):
                        for sub in range(2):
                            jl = half * 2 + sub
                            nc.tensor.matmul(
                                pv[:, sub],
                                xTq_t[:, ct, jl * 128:(jl + 1) * 128],
                                wv_bf[:, ct, :],
                                start=(ct == 0), stop=(ct == NCT - 1),
                            )
                    if part == 3:
                        for sub in range(2):
                            tt = r * 4 + half * 2 + sub
                            nc.vector.tensor_copy(V[:, tt, :, 0:64], pv[:, sub])

                for part in range(4):
                    fillq.append(lambda part=part, c=chunk: c(part))

        def normalize_g(g, att, avc, dn_sb, rc_dr):
            rc_sb = work.tile([128, 8], F32, tag="rc_sb", bufs=4,
                              name="rc_sb")
            nc.vector.reciprocal(rc_sb, dn_sb)
            nc.sync.dma_start(
                out=bass.AP(rc_dr.tensor, rc_dr.offset + g * 1024,
                            [[8, 128], [1, 8]]),
                in_=rc_sb,
            )
            for hh in range(2):
                rep = work.tile([64, 512], F32, tag="rep", bufs=4,
                                name="rep")
                nc.sync.dma_start(
                    out=rep,
                    in_=bass.AP(rc_dr.tensor,
                                rc_dr.offset + (2 * g + hh) * 512,
                                [[0, 64], [1, 512]]),
                )
                if hh == 0:
                    nc.vector.tensor_mul(att[0:64, :], avc[0:64, 0:512], rep)
                else:
                    tmpB = work.tile([64, 512], BF, tag="tmpB", bufs=2,
                                     name="tmpB")
                    nc.vector.tensor_mul(tmpB, avc[0:64, 512:1024], rep)
                    nc.sync.dma_start(out=att[64:128, :], in_=tmpB)

        def push_normalize(r, state, rc_dr):
            att_tiles, avcs = state
            for g in range(NG):
                fillq.append(lambda g=g: normalize_g(
                    g, att_tiles[g], avcs[g][0], avcs[g][1], rc_dr))

        def push_outproj(r, att_tiles):
            for qtl in range(4):
                psy = ps.tile([128, 1024], F32, tag="pp", name=f"psy{r}{qtl}")

                def chunk(part, qtl=qtl, psy=psy):
                    for g in (2 * part, 2 * part + 1):
                        for hf in range(2):
                            nc.tensor.matmul(
                                psy[:, hf * 512:(hf + 1) * 512],
                                att_tiles[g][:, qtl * 128:(qtl + 1) * 128],
                                wo_bf[:, g, hf * 512:(hf + 1) * 512],
                                start=(g == 0), stop=(g == NG - 1),
                            )
                    if part == 1:
                        qt = r * 4 + qtl
                        y_sb = work.tile([128, C], F32, tag="y_sb", bufs=2,
                                         name="y_sb")
                        nc.vector.tensor_copy(y_sb, psy)
                        nc.sync.dma_start(
                            out=y_d.ap()[qt * 128:(qt + 1) * 128, :], in_=y_sb)

                for part in range(2):
                    fillq.append(lambda part=part, c=chunk: c(part))

        def attention_round(r, qq_tiles, dn_dr, rc_dr, pump_rate, start_pump,
                            inline_norm):
            qb = r
            nkt = 4 * (qb + 1)
            att_tiles = []
            avcs = []
            step = 0
            for g in range(NG):
                h0, h1 = 2 * g, 2 * g + 1
                av = ps.tile([65, 1024], F32, tag="av", name=f"av{r}{g}")
                avA = av[:, 0:512]
                avB = av[:, 512:1024]
                att = work.tile([128, 512], BF, tag=f"att{g}", bufs=2,
                                name=f"att{g}")
                qq = qq_tiles[g]
                pend = None
                for kt in range(nkt + 1):
                    if kt < nkt:
                        j = kt - 4 * qb
                        n0 = 128 * j if j > 0 else 0
                        sc = ps.tile([128, 1024], F32, tag="sc", bufs=2,
                                     name="sc")
                        nc.tensor.matmul(
                            sc[:, n0:512],
                            kT[g][0:64, kt * 128:(kt + 1) * 128],
                            qq[0:64, n0:512],
                            start=True, stop=True, tile_position=(0, 0),
                        )
                        nc.tensor.matmul(
                            sc[:, 512 + n0:1024],
                            kT[g][64:128, kt * 128:(kt + 1) * 128],
                            qq[64:128, n0:512],
                            start=True, stop=True, tile_position=(64, 0),
                        )
                        wT = work.tile([128, 1024], BF, tag="wT", bufs=3,
                                       name="wT")
                        if n0 > 0:
                            nc.scalar.activation(wT[:, n0:512], sc[:, n0:512],
                                                 EXP, scale=SCALE)
                            nc.scalar.activation(wT[:, 512 + n0:1024],
                                                 sc[:, 512 + n0:1024],
                                                 EXP, scale=SCALE)
                        else:
                            nc.scalar.activation(wT, sc, EXP, scale=SCALE)
                        if j >= 0:  # diagonal block: triangular causal select
                            for base_col in (n0, 512 + n0):
                                nc.gpsimd.affine_select(
                                    out=wT[:, base_col:base_col + 128],
                                    in_=wT[:, base_col:base_col + 128],
                                    compare_op=mybir.AluOpType.is_ge,
                                    fill=0.0, base=0,
                                    pattern=[[1, 128]],
                                    channel_multiplier=-1,
                                )
                        cur = (wT, kt, n0)
                    if pend is not None:
                        wTp, ktp, n0p = pend
                        nc.tensor.matmul(
                            av[:, n0p:512], V[:, ktp, h0, :],
                            wTp[:, n0p:512],
                            start=(ktp == 0), stop=(ktp == nkt - 1),
                        )
                        nc.tensor.matmul(
                            av[:, 512 + n0p:1024], V[:, ktp, h1, :],
                            wTp[:, 512 + n0p:1024],
                            start=(ktp == 0), stop=(ktp == nkt - 1),
                        )
                        if step >= start_pump:
                            pump(pump_rate)
                        step += 1
                    if kt < nkt:
                        pend = cur
                # drain AV + stage denominators to DRAM for batched recip
                avc = work.tile([65, 1024], F32, tag="avc", bufs=4,
                                name="avc")
                if inline_norm and g == NG - 1:
                    # last block of the kernel: stage the denominator row via
                    # ScalarE straight out of PSUM (ACT is done with exps by
                    # now) so the recip chain starts without waiting for the
                    # full [65,1024] DVE drain.
                    dn_row = work.tile([1, 1024], F32, tag="dn_row",
                                       name="dn_row")
                    nc.scalar.copy(dn_row, av[64:65, :])
                    nc.sync.dma_start(out=dn_dr[g:g + 1, :], in_=dn_row)
                    nc.vector.tensor_copy(avc, av)
                else:
                    nc.vector.tensor_copy(avc, av)
                    nc.sync.dma_start(out=dn_dr[g:g + 1, :],
                                      in_=avc[64:65, :])
                dn_sb = work.tile([128, 8], F32, tag="dn_sb", bufs=4,
                                  name="dn_sb")
                nc.sync.dma_start(
                    out=dn_sb,
                    in_=bass.AP(dn_dr.tensor, dn_dr.offset + g * 1024,
                                [[8, 128], [1, 8]]),
                )
                att_tiles.append(att)
                avcs.append((avc, dn_sb))
                if inline_norm:
                    normalize_g(g, att, avc, dn_sb, rc_dr)
            return att_tiles, avcs

        # ---- main pipeline over T-quarters ----
        qq_tiles = {0: [work.tile([128, 512], BF, tag=f"qq{g}", bufs=2,
                                  name=f"qq{g}") for g in range(NG)]}
        push_qk_proj(0, xTq_t[0], qq_tiles[0])
        push_v_proj(0, xTq_t[0])
        pump(len(fillq))  # round-0 projections emitted inline

        states = {}
        rc_ds = {}
        pump_rates = {0: 2, 1: 2, 2: 1, 3: 1}
        start_pumps = {0: 0, 1: 0, 2: 0, 3: 6}
        for r in range(4):
            dn_dr = dpool.tile([4, 1024], F32, tag="dn_d", bufs=2,
                               name=f"dn_d{r}")
            rc_ds[r] = dpool.tile([4, 1024], F32, tag="rc_d", bufs=2,
                                  name=f"rc_d{r}")
            if r < 3:
                qq_tiles[r + 1] = [work.tile([128, 512], BF, tag=f"qq{g}",
                                             bufs=2, name=f"qq{g}")
                                   for g in range(NG)]
                push_qk_proj(r + 1, xTq_t[r + 1], qq_tiles[r + 1])
                push_v_proj(r + 1, xTq_t[r + 1])
            if r > 0:
                push_normalize(r - 1, states[r - 1], rc_ds[r - 1])
                push_outproj(r - 1, states[r - 1][0])
            states[r] = attention_round(r, qq_tiles[r], dn_dr, rc_ds[r],
                                        pump_rates[r], start_pumps[r],
                                        inline_norm=(r == 3))
            pump(len(fillq))  # drain leftovers at round boundary
        push_outproj(3, states[3][0])
        pump(len(fillq))

    nc.compile()
    return nc


_NC_CACHE = None


def _get_nc():
    global _NC_CACHE
    if _NC_CACHE is None:
        _NC_CACHE = build_nc()
    return _NC_CACHE


def kernel(x, w_qkv, w_out, _trace=False):
    import ml_dtypes

    BF_NP = ml_dtypes.bfloat16
    B = x.shape[0]
    # bf16 on the host: the kernel computes in bf16 anyway, and this halves
    # the HBM upload and removes all on-device casts.
    x = np.asarray(x, dtype=np.float32).astype(BF_NP)
    w_qkv = np.asarray(w_qkv, dtype=np.float32).astype(BF_NP)
    w_out = np.asarray(w_out, dtype=np.float32).astype(BF_NP)

    nc = _get_nc()
    in_maps = []
    for core in range(8):
        b = core % B
        hbase = (core // B) * HC
        lo, hi = hbase * D, hbase * D + HC * D
        in_maps.append({
            "x": np.ascontiguousarray(x[b]),
            "wq": np.ascontiguousarray(w_qkv[:, lo:hi]),
            "wk": np.ascontiguousarray(w_qkv[:, C + lo:C + hi]),
            "wv": np.ascontiguousarray(w_qkv[:, 2 * C + lo:2 * C + hi]),
            "wo": np.ascontiguousarray(w_out[lo:hi, :]),
        })

    res = run_bass_kernel_spmd(nc, in_maps, core_ids=list(range(8)), trace=_trace)
    ys = [r["y"] for r in res.results]
    out = np.empty((B, T, C), dtype=np.float32)
    for b in range(B):
        out[b] = ys[b] + ys[b + B]
    if _trace:
        return out, res
    return out


# revision 17
# speedup vs baseline: 1.6960x; 1.0542x over previous
"""Causal self-attention for trn2, 8 NeuronCores.

Problem: x[4,2048,1024] @ w_qkv[1024,3072] -> causal MHA (16 heads, d=64)
-> @ w_out[1024,1024].

Sharding: core c handles batch b=c%4 and heads hbase=8*(c//4)..hbase+8
(data parallel on B x tensor parallel on heads). Each core computes the
partial out-projection y_c = att_slice @ w_out[slice]; the host sums the
two partials per batch.

v8. Inputs are pre-cast to bf16 on the host (halves HBM upload, removes
all on-device casts). Per round r (T-quarter): project qT/kT/V for
quarter r, run causal attention of q-block r against k-quarters <= r,
out-project. Structure on top of that:
- Two heads per group run score matmuls concurrently on PE row-groups
  (0,0)/(64,0); one [128,1024] f32 PSUM score tile and one exp per
  k-tile step covers both heads. Diagonal k-tiles trim N causally.
- Softmax denominator rides as V's fused ones-column (AV row 64);
  reciprocals batch per head-pair as [128,8] DVE ops via a DRAM gather.
- All projection/out-projection matmuls are emitted in small chunks
  interleaved between attention steps (filler queue) so the PE always
  has ready work while ScalarE paces the exps. Out-projections of
  rounds 0-2 are deferred into round 3, which is otherwise exp-bound.
- Attention inner loop is software-pipelined: scores(kt+1) issue before
  AV(kt) to hide the exp latency.
- Rounds 0-1 get xT via PE transposes fed by small bf16 x loads
  (fast startup); rounds 2-3 via hardware DMA-transpose straight off x.
"""

import sys

for p in ("/opt/trn_rl_repo", "/opt/pypackages"):
    if p not in sys.path:
        sys.path.insert(0, p)

import contextlib
from collections import deque

import numpy as np

import concourse.bass as bass
import concourse.mybir as mybir
import concourse.tile as tile
from concourse import bacc
from concourse.bass_utils import run_bass_kernel_spmd
from concourse.masks import make_identity

F32 = mybir.dt.float32
BF = mybir.dt.bfloat16
EXP = mybir.ActivationFunctionType.Exp

T = 2048          # sequence length
C = 1024          # model dim
HC = 8            # heads per core
D = 64            # head dim
NG = 4            # head-groups of 2 per core
NCT = C // 128    # 8 contraction tiles
NTT = T // 128    # 16 token tiles
SCALE = 0.125     # 1/sqrt(D)


def build_nc():
    nc = bacc.Bacc("TRN2", target_bir_lowering=False, debug=False)

    x_d = nc.dram_tensor("x", [T, C], BF, kind="ExternalInput")
    wq_d = nc.dram_tensor("wq", [C, 512], BF, kind="ExternalInput")
    wk_d = nc.dram_tensor("wk", [C, 512], BF, kind="ExternalInput")
    wv_d = nc.dram_tensor("wv", [C, 512], BF, kind="ExternalInput")
    wo_d = nc.dram_tensor("wo", [512, C], BF, kind="ExternalInput")
    y_d = nc.dram_tensor("y", [T, C], F32, kind="ExternalOutput")

    with tile.TileContext(nc) as tc, contextlib.ExitStack() as ctx:
        persist = ctx.enter_context(tc.tile_pool(name="persist", bufs=1))
        work = ctx.enter_context(tc.tile_pool(name="work", bufs=1))
        ps = ctx.enter_context(tc.tile_pool(name="ps", bufs=1, space="PSUM"))
        dpool = ctx.enter_context(tc.tile_pool(name="dram", bufs=1, space="DRAM"))

        kT = [persist.tile([128, T], BF, tag=f"kT{g}", name=f"kT{g}")
              for g in range(NG)]
        V = persist.tile([128, NTT, HC, 65], BF, tag="V")

        wq_bf = persist.tile([128, NCT, 512], BF, tag="wq_bf")
        wk_bf = persist.tile([128, NCT, 512], BF, tag="wk_bf")
        wv_bf = persist.tile([128, NCT, 512], BF, tag="wv_bf")
        wo_bf = persist.tile([128, NG, C], BF, tag="wo_bf")

        ident = persist.tile([128, 128], BF, tag="ident", name="ident")
        make_identity(nc, ident)

        # small bf16 x loads first so PE transposes start ~2us in
        x_nats = []
        for j in range(8):
            x_nat = work.tile([128, C], BF, tag="x_nat", bufs=4, name="x_nat")
            nc.sync.dma_start(out=x_nat, in_=x_d.ap()[j * 128:(j + 1) * 128, :])
            x_nats.append(x_nat)
            if j == 0:
                nc.sync.dma_start(
                    out=wq_bf,
                    in_=wq_d.ap().rearrange("(ct p) m -> p ct m", p=128))
            elif j == 2:
                nc.sync.dma_start(
                    out=wk_bf,
                    in_=wk_d.ap().rearrange("(ct p) m -> p ct m", p=128))
            elif j == 4:
                nc.sync.dma_start(
                    out=wv_bf,
                    in_=wv_d.ap().rearrange("(ct p) m -> p ct m", p=128))
            elif j == 6:
                nc.sync.dma_start(
                    out=wo_bf,
                    in_=wo_d.ap().rearrange("(g p) c -> p g c", p=128))

        xTq_t = {
            0: work.tile([128, NCT, 512], BF, tag="xTq", bufs=4, name="xTq0"),
            1: work.tile([128, NCT, 512], BF, tag="xTq", bufs=4, name="xTq1"),
        }
        for j in range(8):
            dst = xTq_t[0] if j < 4 else xTq_t[1]
            jj = j % 4
            tp0 = ps.tile([128, NCT, 128], BF, tag="sc", bufs=2, name="tp0")
            for ct in range(NCT):
                nc.tensor.transpose(tp0[:, ct, :],
                                    x_nats[j][:, ct * 128:(ct + 1) * 128],
                                    ident)
            nc.vector.tensor_copy(dst[:, :, jj * 128:(jj + 1) * 128], tp0)

        # rounds 2-3: hardware DMA-transpose straight off bf16 x
        for rr in (2, 3):
            t = work.tile([128, NCT, 512], BF, tag="xTq", bufs=4,
                          name=f"xTq{rr}")
            xTq_t[rr] = t
            q0 = rr * 512
            for ct in range(NCT):
                nc.sync.dma_start_transpose(
                    out=t[:, ct, :],
                    in_=x_d.ap()[q0:q0 + 512, ct * 128:(ct + 1) * 128])

        # ones column of V (AV matmul row 64 = softmax denominator)
        ones_f32 = persist.tile([128, NTT, HC], F32, tag="ones")
        nc.vector.memset(ones_f32, 1.0)
        nc.vector.tensor_copy(V[:, :, :, 64], ones_f32)

        # ---- filler queue: deferred PE work interleaved into attention ----
        fillq = deque()
        pump_acc = [0.0]

        def pump(rate):
            pump_acc[0] += rate
            while fillq and pump_acc[0] >= 1.0:
                fillq.popleft()()
                pump_acc[0] -= 1.0

        def drain():
            while fillq:
                fillq.popleft()()

        def qk_proj_chunks(r, xTq_q, qq_tiles):
            out = []
            for g in range(NG):
                pqk = ps.tile([128, 1024], F32, tag="pp", name=f"pqk{r}{g}")

                def chunk(part, g=g, pqk=pqk, qq=qq_tiles[g]):
                    wbf = wq_bf if part < 2 else wk_bf
                    osl = slice(0, 512) if part < 2 else slice(512, 1024)
                    cts = range(0, 4) if part % 2 == 0 else range(4, 8)
                    for ct in cts:
                        nc.tensor.matmul(
                            pqk[:, osl],
                            wbf[:, ct, g * 128:(g + 1) * 128],
                            xTq_q[:, ct, :],
                            start=(ct == 0), stop=(ct == NCT - 1),
                        )
                    if part == 1:
                        nc.vector.tensor_copy(qq, pqk[:, 0:512])
                    elif part == 3:
                        nc.vector.tensor_copy(
                            kT[g][:, r * 512:(r + 1) * 512], pqk[:, 512:1024])

                for part in range(4):
                    out.append(lambda part=part, c=chunk: c(part))
            return out

        def v_proj_chunks(r, xTq_q):
            out = []
            for half in range(2):
                pv = ps.tile([128, 2, HC, 64], F32, tag="pp",
                             name=f"pv{r}{half}")

                def chunk(part, half=half, pv=pv):
                    for ct in (2 * part, 2 * part + 1):
                        for sub in range(2):
                            jl = half * 2 + sub
                            nc.tensor.matmul(
                                pv[:, sub],
                                xTq_q[:, ct, jl * 128:(jl + 1) * 128],
                                wv_bf[:, ct, :],
                                start=(ct == 0), stop=(ct == NCT - 1),
                            )
                    if part == 3:
                        for sub in range(2):
                            tt = r * 4 + half * 2 + sub
                            nc.vector.tensor_copy(V[:, tt, :, 0:64], pv[:, sub])

                for part in range(4):
                    out.append(lambda part=part, c=chunk: c(part))
            return out

        def norm_g_a(g, dn_sb, rc_dr):
            rc_sb = work.tile([128, 8], F32, tag="rc_sb", bufs=4, name="rc_sb")
            nc.vector.reciprocal(rc_sb, dn_sb)
            nc.sync.dma_start(
                out=bass.AP(rc_dr.tensor, rc_dr.offset + g * 1024,
                            [[8, 128], [1, 8]]),
                in_=rc_sb,
            )

        def norm_g_b(g, att, avc, rc_dr):
            for hh in range(2):
                rep = work.tile([64, 512], F32, tag="rep", bufs=4, name="rep")
                nc.sync.dma_start(
                    out=rep,
                    in_=bass.AP(rc_dr.tensor,
                                rc_dr.offset + (2 * g + hh) * 512,
                                [[0, 64], [1, 512]]),
                )
                if hh == 0:
                    nc.vector.tensor_mul(att[0:64, :], avc[0:64, 0:512], rep)
                else:
                    tmpB = work.tile([64, 512], BF, tag="tmpB", bufs=2,
                                     name="tmpB")
                    nc.vector.tensor_mul(tmpB, avc[0:64, 512:1024], rep)
                    nc.sync.dma_start(out=att[64:128, :], in_=tmpB)

        def norm_a_chunks(state, rc_dr):
            att_tiles, avcs = state
            return [lambda g=g: norm_g_a(g, avcs[g][1], rc_dr)
                    for g in range(NG)]

        def norm_b_chunks(state, rc_dr):
            att_tiles, avcs = state
            return [lambda g=g: norm_g_b(g, att_tiles[g], avcs[g][0], rc_dr)
                    for g in range(NG)]

        def outproj_chunks(r, att_tiles):
            out = []
            for qtl in range(4):
                psy = ps.tile([128, 1024], F32, tag="pp", name=f"psy{r}{qtl}")

                def chunk(part, qtl=qtl, psy=psy):
                    for g in (2 * part, 2 * part + 1):
                        for hf in range(2):
                            nc.tensor.matmul(
                                psy[:, hf * 512:(hf + 1) * 512],
                                att_tiles[g][:, qtl * 128:(qtl + 1) * 128],
                                wo_bf[:, g, hf * 512:(hf + 1) * 512],
                                start=(g == 0), stop=(g == NG - 1),
                            )
                    if part == 1:
                        qt = r * 4 + qtl
                        y_sb = work.tile([128, C], F32, tag="y_sb", bufs=2,
                                         name="y_sb")
                        nc.vector.tensor_copy(y_sb, psy)
                        nc.sync.dma_start(
                            out=y_d.ap()[qt * 128:(qt + 1) * 128, :], in_=y_sb)

                for part in range(2):
                    out.append(lambda part=part, c=chunk: c(part))
            return out

        def attention_round(r, qq_tiles, dn_dr, rc_dr, pump_rate, start_pump,
                            inline_norm):
            qb = r
            nkt = 4 * (qb + 1)
            att_tiles = []
            avcs = []
            step = 0
            for g in range(NG):
                h0, h1 = 2 * g, 2 * g + 1
                av = ps.tile([65, 1024], F32, tag="av", name=f"av{r}{g}")
                att = work.tile([128, 512], BF, tag=f"att{g}", bufs=4,
                                name=f"att{g}")
                qq = qq_tiles[g]
                pend = None
                for kt in range(nkt + 1):
                    if kt < nkt:
                        j = kt - 4 * qb
                        n0 = 128 * j if j > 0 else 0
                        sc = ps.tile([128, 1024], F32, tag="sc", bufs=2,
                                     name="sc")
                        nc.tensor.matmul(
                            sc[:, n0:512],
                            kT[g][0:64, kt * 128:(kt + 1) * 128],
                            qq[0:64, n0:512],
                            start=True, stop=True, tile_position=(0, 0),
                        )
                        nc.tensor.matmul(
                            sc[:, 512 + n0:1024],
                            kT[g][64:128, kt * 128:(kt + 1) * 128],
                            qq[64:128, n0:512],
                            start=True, stop=True, tile_position=(64, 0),
                        )
                        wT = work.tile([128, 1024], BF, tag="wT", bufs=3,
                                       name="wT")
                        if n0 > 0:
                            nc.scalar.activation(wT[:, n0:512], sc[:, n0:512],
                                                 EXP, scale=SCALE)
                            nc.scalar.activation(wT[:, 512 + n0:1024],
                                                 sc[:, 512 + n0:1024],
                                                 EXP, scale=SCALE)
                        else:
                            nc.scalar.activation(wT, sc, EXP, scale=SCALE)
                        if j >= 0:  # diagonal block: triangular causal select
                            for base_col in (n0, 512 + n0):
                                nc.gpsimd.affine_select(
                                    out=wT[:, base_col:base_col + 128],
                                    in_=wT[:, base_col:base_col + 128],
                                    compare_op=mybir.AluOpType.is_ge,
                                    fill=0.0, base=0,
                                    pattern=[[1, 128]],
                                    channel_multiplier=-1,
                                )
                        cur = (wT, kt, n0)
                    if pend is not None:
                        wTp, ktp, n0p = pend
                        nc.tensor.matmul(
                            av[:, n0p:512], V[:, ktp, h0, :],
                            wTp[:, n0p:512],
                            start=(ktp == 0), stop=(ktp == nkt - 1),
                        )
                        nc.tensor.matmul(
                            av[:, 512 + n0p:1024], V[:, ktp, h1, :],
                            wTp[:, 512 + n0p:1024],
                            start=(ktp == 0), stop=(ktp == nkt - 1),
                        )
                        if step >= start_pump:
                            pump(pump_rate)
                        step += 1
                    if kt < nkt:
                        pend = cur
                # drain AV + stage denominators to DRAM for batched recip
                avc = work.tile([65, 1024], F32, tag="avc", bufs=4,
                                name="avc")
                if inline_norm and g == NG - 1:
                    # last block of the kernel: stage the denominator row via
                    # ScalarE straight out of PSUM so the recip chain starts
                    # without waiting for the full [65,1024] DVE drain.
                    dn_row = work.tile([1, 1024], F32, tag="dn_row",
                                       name="dn_row")
                    nc.scalar.copy(dn_row, av[64:65, :])
                    nc.sync.dma_start(out=dn_dr[g:g + 1, :], in_=dn_row)
                    nc.vector.tensor_copy(avc, av)
                else:
                    nc.vector.tensor_copy(avc, av)
                    nc.sync.dma_start(out=dn_dr[g:g + 1, :],
                                      in_=avc[64:65, :])
                dn_sb = work.tile([128, 8], F32, tag="dn_sb", bufs=4,
                                  name="dn_sb")
                nc.sync.dma_start(
                    out=dn_sb,
                    in_=bass.AP(dn_dr.tensor, dn_dr.offset + g * 1024,
                                [[8, 128], [1, 8]]),
                )
                att_tiles.append(att)
                avcs.append((avc, dn_sb))
                if inline_norm:
                    norm_g_a(g, dn_sb, rc_dr)
                    norm_g_b(g, att, avc, rc_dr)
            return att_tiles, avcs

        # ---- main pipeline over T-quarters ----
        def mk_qq():
            return [work.tile([128, 512], BF, tag=f"qq{g}", bufs=2,
                              name=f"qq{g}") for g in range(NG)]

        qq_tiles = {0: mk_qq()}
        fillq.extend(qk_proj_chunks(0, xTq_t[0], qq_tiles[0]))
        fillq.extend(v_proj_chunks(0, xTq_t[0]))
        drain()  # round-0 projections emitted inline

        states = {}
        rc_ds = {}
        dn_ds = {}
        pump_rates = {0: 2.0, 1: 1.0, 2: 0.7, 3: 0.55}
        start_pumps = {0: 0, 1: 0, 2: 0, 3: 2}
        for r in range(4):
            dn_ds[r] = dpool.tile([4, 1024], F32, tag="dn_d", bufs=2,
                                  name=f"dn_d{r}")
            rc_ds[r] = dpool.tile([4, 1024], F32, tag="rc_d", bufs=2,
                                  name=f"rc_d{r}")
            # build this round's filler queue
            if r < 3:
                qq_tiles[r + 1] = mk_qq()
                proj = (qk_proj_chunks(r + 1, xTq_t[r + 1], qq_tiles[r + 1])
                        + v_proj_chunks(r + 1, xTq_t[r + 1]))
            else:
                proj = []
            if r == 0:
                fillq.extend(proj)
            elif r in (1, 2):
                fillq.extend(norm_a_chunks(states[r - 1], rc_ds[r - 1]))
                fillq.extend(proj[:4])
                fillq.extend(norm_b_chunks(states[r - 1], rc_ds[r - 1]))
                fillq.extend(proj[4:])
            else:  # r == 3: fill the exp-bound round with all out-projections
                fillq.extend(norm_a_chunks(states[2], rc_ds[2]))
                fillq.extend(outproj_chunks(0, states[0][0]))
                fillq.extend(norm_b_chunks(states[2], rc_ds[2]))
                fillq.extend(outproj_chunks(1, states[1][0]))
                fillq.extend(outproj_chunks(2, states[2][0]))
            states[r] = attention_round(r, qq_tiles[r], dn_ds[r], rc_ds[r],
                                        pump_rates[r], start_pumps[r],
                                        inline_norm=(r == 3))
            drain()  # leftovers at the round boundary
        fillq.extend(outproj_chunks(3, states[3][0]))
        drain()

    nc.compile()
    return nc


_NC_CACHE = None


def _get_nc():
    global _NC_CACHE
    if _NC_CACHE is None:
        _NC_CACHE = build_nc()
    return _NC_CACHE


def kernel(x, w_qkv, w_out, _trace=False):
    import ml_dtypes

    BF_NP = ml_dtypes.bfloat16
    B = x.shape[0]
    # bf16 on the host: the kernel computes in bf16 anyway, and this halves
    # the HBM upload and removes all on-device casts.
    x = np.asarray(x, dtype=np.float32).astype(BF_NP)
    w_qkv = np.asarray(w_qkv, dtype=np.float32).astype(BF_NP)
    w_out = np.asarray(w_out, dtype=np.float32).astype(BF_NP)

    nc = _get_nc()
    in_maps = []
    for core in range(8):
        b = core % B
        hbase = (core // B) * HC
        lo, hi = hbase * D, hbase * D + HC * D
        in_maps.append({
            "x": np.ascontiguousarray(x[b]),
            "wq": np.ascontiguousarray(w_qkv[:, lo:hi]),
            "wk": np.ascontiguousarray(w_qkv[:, C + lo:C + hi]),
            "wv": np.ascontiguousarray(w_qkv[:, 2 * C + lo:2 * C + hi]),
            "wo": np.ascontiguousarray(w_out[lo:hi, :]),
        })

    res = run_bass_kernel_spmd(nc, in_maps, core_ids=list(range(8)), trace=_trace)
    ys = [r["y"] for r in res.results]
    out = np.empty((B, T, C), dtype=np.float32)
    for b in range(B):
        out[b] = ys[b] + ys[b + B]
    if _trace:
        return out, res
    return out


# revision 21
# speedup vs baseline: 1.7162x; 1.0119x over previous
"""Causal self-attention for trn2, 8 NeuronCores.

Problem: x[4,2048,1024] @ w_qkv[1024,3072] -> causal MHA (16 heads, d=64)
-> @ w_out[1024,1024].

Sharding: core c handles batch b=c%4 and heads hbase=8*(c//4)..hbase+8
(data parallel on B x tensor parallel on heads). Each core computes the
partial out-projection y_c = att_slice @ w_out[slice]; the host sums the
two partials per batch.

v8. Inputs are pre-cast to bf16 on the host (halves HBM upload, removes
all on-device casts). Per round r (T-quarter): project qT/kT/V for
quarter r, run causal attention of q-block r against k-quarters <= r,
out-project. Structure on top of that:
- Two heads per group run score matmuls concurrently on PE row-groups
  (0,0)/(64,0); one [128,1024] f32 PSUM score tile and one exp per
  k-tile step covers both heads. Diagonal k-tiles trim N causally.
- Softmax denominator rides as V's fused ones-column (AV row 64);
  reciprocals batch per head-pair as [128,8] DVE ops via a DRAM gather.
- All projection/out-projection matmuls are emitted in small chunks
  interleaved between attention steps (filler queue) so the PE always
  has ready work while ScalarE paces the exps. Out-projections of
  rounds 0-2 are deferred into round 3, which is otherwise exp-bound.
- Attention inner loop is software-pipelined: scores(kt+1) issue before
  AV(kt) to hide the exp latency.
- Rounds 0-1 get xT via PE transposes fed by small bf16 x loads
  (fast startup); rounds 2-3 via hardware DMA-transpose straight off x.
"""

import sys

for p in ("/opt/trn_rl_repo", "/opt/pypackages"):
    if p not in sys.path:
        sys.path.insert(0, p)

import contextlib
from collections import deque

import numpy as np

import concourse.bass as bass
import concourse.mybir as mybir
import concourse.tile as tile
from concourse import bacc
from concourse.bass_utils import run_bass_kernel_spmd
from concourse.masks import make_identity

F32 = mybir.dt.float32
BF = mybir.dt.bfloat16
EXP = mybir.ActivationFunctionType.Exp

T = 2048          # sequence length
C = 1024          # model dim
HC = 8            # heads per core
D = 64            # head dim
NG = 4            # head-groups of 2 per core
NCT = C // 128    # 8 contraction tiles
NTT = T // 128    # 16 token tiles
SCALE = 0.125     # 1/sqrt(D)


def build_nc():
    nc = bacc.Bacc("TRN2", target_bir_lowering=False, debug=False)

    x_d = nc.dram_tensor("x", [T, C], BF, kind="ExternalInput")
    wq_d = nc.dram_tensor("wq", [C, 512], BF, kind="ExternalInput")
    wk_d = nc.dram_tensor("wk", [C, 512], BF, kind="ExternalInput")
    wv_d = nc.dram_tensor("wv", [C, 512], BF, kind="ExternalInput")
    wo_d = nc.dram_tensor("wo", [512, C], BF, kind="ExternalInput")
    y_d = nc.dram_tensor("y", [T, C], F32, kind="ExternalOutput")

    with tile.TileContext(nc) as tc, contextlib.ExitStack() as ctx:
        persist = ctx.enter_context(tc.tile_pool(name="persist", bufs=1))
        work = ctx.enter_context(tc.tile_pool(name="work", bufs=1))
        ps = ctx.enter_context(tc.tile_pool(name="ps", bufs=1, space="PSUM"))
        dpool = ctx.enter_context(tc.tile_pool(name="dram", bufs=1, space="DRAM"))

        kT = [persist.tile([128, T], BF, tag=f"kT{g}", name=f"kT{g}")
              for g in range(NG)]
        V = persist.tile([128, NTT, HC, 65], BF, tag="V")

        wq_bf = persist.tile([128, NCT, 512], BF, tag="wq_bf")
        wk_bf = persist.tile([128, NCT, 512], BF, tag="wk_bf")
        wv_bf = persist.tile([128, NCT, 512], BF, tag="wv_bf")
        wo_bf = persist.tile([128, NG, C], BF, tag="wo_bf")

        ident = persist.tile([128, 128], BF, tag="ident", name="ident")
        make_identity(nc, ident)

        # small bf16 x loads first so PE transposes start ~2us in
        x_nats = []
        for j in range(8):
            x_nat = work.tile([128, C], BF, tag="x_nat", bufs=4, name="x_nat")
            nc.sync.dma_start(out=x_nat, in_=x_d.ap()[j * 128:(j + 1) * 128, :])
            x_nats.append(x_nat)
            if j == 0:
                nc.sync.dma_start(
                    out=wq_bf,
                    in_=wq_d.ap().rearrange("(ct p) m -> p ct m", p=128))
            elif j == 2:
                nc.sync.dma_start(
                    out=wk_bf,
                    in_=wk_d.ap().rearrange("(ct p) m -> p ct m", p=128))
        nc.sync.dma_start(
            out=wv_bf, in_=wv_d.ap().rearrange("(ct p) m -> p ct m", p=128))

        xTq_t = {
            0: work.tile([128, NCT, 512], BF, tag="xTq", bufs=4, name="xTq0"),
            1: work.tile([128, NCT, 512], BF, tag="xTq", bufs=4, name="xTq1"),
        }
        for j in range(8):
            dst = xTq_t[0] if j < 4 else xTq_t[1]
            jj = j % 4
            tp0 = ps.tile([128, NCT, 128], BF, tag="sc", bufs=2, name="tp0")
            for ct in range(NCT):
                nc.tensor.transpose(tp0[:, ct, :],
                                    x_nats[j][:, ct * 128:(ct + 1) * 128],
                                    ident)
            nc.vector.tensor_copy(dst[:, :, jj * 128:(jj + 1) * 128], tp0)

        # rounds 2-3: hardware DMA-transpose straight off bf16 x
        for rr in (2, 3):
            t = work.tile([128, NCT, 512], BF, tag="xTq", bufs=4,
                          name=f"xTq{rr}")
            xTq_t[rr] = t
            q0 = rr * 512
            for ct in range(NCT):
                nc.sync.dma_start_transpose(
                    out=t[:, ct, :],
                    in_=x_d.ap()[q0:q0 + 512, ct * 128:(ct + 1) * 128])
        nc.sync.dma_start(
            out=wo_bf, in_=wo_d.ap().rearrange("(g p) c -> p g c", p=128))

        # ones column of V (AV matmul row 64 = softmax denominator)
        ones_f32 = persist.tile([128, NTT, HC], F32, tag="ones")
        nc.vector.memset(ones_f32, 1.0)
        nc.vector.tensor_copy(V[:, :, :, 64], ones_f32)

        # ---- filler queue: deferred PE work interleaved into attention ----
        fillq = deque()
        pump_acc = [0.0]

        def pump(rate):
            pump_acc[0] += rate
            while fillq and pump_acc[0] >= 1.0:
                fillq.popleft()()
                pump_acc[0] -= 1.0

        def drain():
            while fillq:
                fillq.popleft()()

        def qk_proj_chunks(r, xTq_q, qq_tiles):
            out = []
            for g in range(NG):
                pqk = ps.tile([128, 1024], F32, tag="pp", name=f"pqk{r}{g}")

                def chunk(part, g=g, pqk=pqk, qq=qq_tiles[g]):
                    wbf = wq_bf if part < 2 else wk_bf
                    osl = slice(0, 512) if part < 2 else slice(512, 1024)
                    cts = range(0, 4) if part % 2 == 0 else range(4, 8)
                    for ct in cts:
                        nc.tensor.matmul(
                            pqk[:, osl],
                            wbf[:, ct, g * 128:(g + 1) * 128],
                            xTq_q[:, ct, :],
                            start=(ct == 0), stop=(ct == NCT - 1),
                        )
                    if part == 1:
                        nc.vector.tensor_copy(qq, pqk[:, 0:512])
                    elif part == 3:
                        nc.vector.tensor_copy(
                            kT[g][:, r * 512:(r + 1) * 512], pqk[:, 512:1024])

                for part in range(4):
                    out.append(lambda part=part, c=chunk: c(part))
            return out

        def v_proj_chunks(r, xTq_q):
            out = []
            for half in range(2):
                pv = ps.tile([128, 2, HC, 64], F32, tag="pp",
                             name=f"pv{r}{half}")

                def chunk(part, half=half, pv=pv):
                    for ct in (2 * part, 2 * part + 1):
                        for sub in range(2):
                            jl = half * 2 + sub
                            nc.tensor.matmul(
                                pv[:, sub],
                                xTq_q[:, ct, jl * 128:(jl + 1) * 128],
                                wv_bf[:, ct, :],
                                start=(ct == 0), stop=(ct == NCT - 1),
                            )
                    if part == 3:
                        for sub in range(2):
                            tt = r * 4 + half * 2 + sub
                            nc.vector.tensor_copy(V[:, tt, :, 0:64], pv[:, sub])

                for part in range(4):
                    out.append(lambda part=part, c=chunk: c(part))
            return out

        def norm_g_a(g, dn_sb, rc_dr):
            rc_sb = work.tile([128, 8], F32, tag="rc_sb", bufs=4, name="rc_sb")
            nc.vector.reciprocal(rc_sb, dn_sb)
            nc.sync.dma_start(
                out=bass.AP(rc_dr.tensor, rc_dr.offset + g * 1024,
                            [[8, 128], [1, 8]]),
                in_=rc_sb,
            )

        def norm_g_b(g, att, avc, rc_dr):
            for hh in range(2):
                rep = work.tile([64, 512], F32, tag="rep", bufs=4, name="rep")
                nc.sync.dma_start(
                    out=rep,
                    in_=bass.AP(rc_dr.tensor,
                                rc_dr.offset + (2 * g + hh) * 512,
                                [[0, 64], [1, 512]]),
                )
                if hh == 0:
                    nc.vector.tensor_mul(att[0:64, :], avc[0:64, 0:512], rep)
                else:
                    tmpB = work.tile([64, 512], BF, tag="tmpB", bufs=2,
                                     name="tmpB")
                    nc.vector.tensor_mul(tmpB, avc[0:64, 512:1024], rep)
                    nc.sync.dma_start(out=att[64:128, :], in_=tmpB)

        def norm_a_chunks(state, rc_dr):
            att_tiles, avcs = state
            return [lambda g=g: norm_g_a(g, avcs[g][1], rc_dr)
                    for g in range(NG)]

        def norm_b_chunks(state, rc_dr):
            att_tiles, avcs = state
            return [lambda g=g: norm_g_b(g, att_tiles[g], avcs[g][0], rc_dr)
                    for g in range(NG)]

        def outproj_chunks(r, att_tiles):
            out = []
            for qtl in range(4):
                psy = ps.tile([128, 1024], F32, tag="pp", name=f"psy{r}{qtl}")

                def chunk(part, qtl=qtl, psy=psy):
                    for g in (2 * part, 2 * part + 1):
                        for hf in range(2):
                            nc.tensor.matmul(
                                psy[:, hf * 512:(hf + 1) * 512],
                                att_tiles[g][:, qtl * 128:(qtl + 1) * 128],
                                wo_bf[:, g, hf * 512:(hf + 1) * 512],
                                start=(g == 0), stop=(g == NG - 1),
                            )
                    if part == 1:
                        qt = r * 4 + qtl
                        y_sb = work.tile([128, C], F32, tag="y_sb", bufs=2,
                                         name="y_sb")
                        nc.vector.tensor_copy(y_sb, psy)
                        nc.sync.dma_start(
                            out=y_d.ap()[qt * 128:(qt + 1) * 128, :], in_=y_sb)

                for part in range(2):
                    out.append(lambda part=part, c=chunk: c(part))
            return out

        def attention_round(r, qq_tiles, dn_dr, rc_dr, pump_rate, start_pump,
                            inline_norm):
            qb = r
            nkt = 4 * (qb + 1)
            att_tiles = []
            avcs = []
            step = 0
            for g in range(NG):
                h0, h1 = 2 * g, 2 * g + 1
                av = ps.tile([65, 1024], F32, tag="av", name=f"av{r}{g}")
                att = work.tile([128, 512], BF, tag=f"att{g}", bufs=4,
                                name=f"att{g}")
                qq = qq_tiles[g]
                pend = None
                for kt in range(nkt + 1):
                    if kt < nkt:
                        j = kt - 4 * qb
                        n0 = 128 * j if j > 0 else 0
                        sc = ps.tile([128, 1024], F32, tag="sc", bufs=2,
                                     name="sc")
                        nc.tensor.matmul(
                            sc[:, n0:512],
                            kT[g][0:64, kt * 128:(kt + 1) * 128],
                            qq[0:64, n0:512],
                            start=True, stop=True, tile_position=(0, 0),
                        )
                        nc.tensor.matmul(
                            sc[:, 512 + n0:1024],
                            kT[g][64:128, kt * 128:(kt + 1) * 128],
                            qq[64:128, n0:512],
                            start=True, stop=True, tile_position=(64, 0),
                        )
                        wT = work.tile([128, 1024], BF, tag="wT", bufs=3,
                                       name="wT")
                        if n0 > 0:
                            nc.scalar.activation(wT[:, n0:512], sc[:, n0:512],
                                                 EXP, scale=SCALE)
                            nc.scalar.activation(wT[:, 512 + n0:1024],
                                                 sc[:, 512 + n0:1024],
                                                 EXP, scale=SCALE)
                        else:
                            nc.scalar.activation(wT, sc, EXP, scale=SCALE)
                        if j >= 0:  # diagonal block: triangular causal select
                            for base_col in (n0, 512 + n0):
                                nc.gpsimd.affine_select(
                                    out=wT[:, base_col:base_col + 128],
                                    in_=wT[:, base_col:base_col + 128],
                                    compare_op=mybir.AluOpType.is_ge,
                                    fill=0.0, base=0,
                                    pattern=[[1, 128]],
                                    channel_multiplier=-1,
                                )
                        cur = (wT, kt, n0)
                    if pend is not None:
                        wTp, ktp, n0p = pend
                        # fillers go between scores(kt) and AV(kt-1) so the
                        # PE has work while ScalarE finishes exp(kt-1)
                        if step >= start_pump:
                            pump(pump_rate)
                        step += 1
                        nc.tensor.matmul(
                            av[:, n0p:512], V[:, ktp, h0, :],
                            wTp[:, n0p:512],
                            start=(ktp == 0), stop=(ktp == nkt - 1),
                        )
                        nc.tensor.matmul(
                            av[:, 512 + n0p:1024], V[:, ktp, h1, :],
                            wTp[:, 512 + n0p:1024],
                            start=(ktp == 0), stop=(ktp == nkt - 1),
                        )
                    if kt < nkt:
                        pend = cur
                # drain AV + stage denominators to DRAM for batched recip
                avc = work.tile([65, 1024], F32, tag="avc", bufs=4,
                                name="avc")
                if inline_norm and g == NG - 1:
                    # Last block of the kernel (the tail chain). Shorten it:
                    # ScalarE stages the denominator row straight out of PSUM,
                    # a single-lane fast-approx reciprocal replaces the
                    # [128,8] DRAM-gather round trip, and ScalarE-paced dummy
                    # matmuls keep the PE clock warm through the chain so the
                    # final out-projection runs at full rate.
                    dn_row = work.tile([1, 1024], F32, tag="dn_row",
                                       name="dn_row")
                    nc.scalar.copy(dn_row, av[64:65, :])
                    rc_row = work.tile([1, 1024], F32, tag="rc_row",
                                       name="rc_row")
                    nc.vector.reciprocal_approx_fast(rc_row, dn_row)
                    nc.sync.dma_start(
                        out=bass.AP(rc_dr.tensor, rc_dr.offset + g * 1024,
                                    [[1, 1], [1, 1024]]),
                        in_=rc_row,
                    )
                    nc.vector.tensor_copy(avc, av)
                    for w in range(6):
                        wsrc = work.tile([128, 512], BF, tag="warm", bufs=2,
                                         name="warm")
                        nc.scalar.copy(wsrc, wo_bf[:, 0, 0:512])
                        wps = ps.tile([128, 512], F32, tag="pp", name="wps")
                        nc.tensor.matmul(wps, wq_bf[:, 0, 0:128], wsrc,
                                         start=True, stop=True)
                    att_tiles.append(att)
                    avcs.append((avc, None))
                    norm_g_b(g, att, avc, rc_dr)
                    continue
                nc.vector.tensor_copy(avc, av)
                nc.sync.dma_start(out=dn_dr[g:g + 1, :], in_=avc[64:65, :])
                dn_sb = work.tile([128, 8], F32, tag="dn_sb", bufs=4,
                                  name="dn_sb")
                nc.sync.dma_start(
                    out=dn_sb,
                    in_=bass.AP(dn_dr.tensor, dn_dr.offset + g * 1024,
                                [[8, 128], [1, 8]]),
                )
                att_tiles.append(att)
                avcs.append((avc, dn_sb))
                if inline_norm:
                    norm_g_a(g, dn_sb, rc_dr)
                    norm_g_b(g, att, avc, rc_dr)
            return att_tiles, avcs

        # ---- main pipeline over T-quarters ----
        def mk_qq():
            return [work.tile([128, 512], BF, tag=f"qq{g}", bufs=2,
                              name=f"qq{g}") for g in range(NG)]

        qq_tiles = {0: mk_qq()}
        fillq.extend(qk_proj_chunks(0, xTq_t[0], qq_tiles[0]))
        fillq.extend(v_proj_chunks(0, xTq_t[0]))
        drain()  # round-0 projections emitted inline

        states = {}
        rc_ds = {}
        dn_ds = {}
        pump_rates = {0: 2.0, 1: 1.0, 2: 0.7, 3: 0.55}
        start_pumps = {0: 0, 1: 0, 2: 0, 3: 2}
        for r in range(4):
            dn_ds[r] = dpool.tile([4, 1024], F32, tag="dn_d", bufs=2,
                                  name=f"dn_d{r}")
            rc_ds[r] = dpool.tile([4, 1024], F32, tag="rc_d", bufs=2,
                                  name=f"rc_d{r}")
            # build this round's filler queue
            if r < 3:
                qq_tiles[r + 1] = mk_qq()
                proj = (qk_proj_chunks(r + 1, xTq_t[r + 1], qq_tiles[r + 1])
                        + v_proj_chunks(r + 1, xTq_t[r + 1]))
            else:
                proj = []
            if r == 0:
                fillq.extend(proj)
            elif r in (1, 2):
                fillq.extend(norm_a_chunks(states[r - 1], rc_ds[r - 1]))
                fillq.extend(proj[:4])
                fillq.extend(norm_b_chunks(states[r - 1], rc_ds[r - 1]))
                fillq.extend(proj[4:])
            else:  # r == 3: fill the exp-bound round with all out-projections
                fillq.extend(norm_a_chunks(states[2], rc_ds[2]))
                fillq.extend(outproj_chunks(0, states[0][0]))
                fillq.extend(norm_b_chunks(states[2], rc_ds[2]))
                fillq.extend(outproj_chunks(1, states[1][0]))
                fillq.extend(outproj_chunks(2, states[2][0]))
            states[r] = attention_round(r, qq_tiles[r], dn_ds[r], rc_ds[r],
                                        pump_rates[r], start_pumps[r],
                                        inline_norm=(r == 3))
            drain()  # leftovers at the round boundary
        fillq.extend(outproj_chunks(3, states[3][0]))
        drain()

    nc.compile()
    return nc


_NC_CACHE = None


def _get_nc():
    global _NC_CACHE
    if _NC_CACHE is None:
        _NC_CACHE = build_nc()
    return _NC_CACHE


def kernel(x, w_qkv, w_out, _trace=False):
    import ml_dtypes

    BF_NP = ml_dtypes.bfloat16
    B = x.shape[0]
    # bf16 on the host: the kernel computes in bf16 anyway, and this halves
    # the HBM upload and removes all on-device casts.
    x = np.asarray(x, dtype=np.float32).astype(BF_NP)
    w_qkv = np.asarray(w_qkv, dtype=np.float32).astype(BF_NP)
    w_out = np.asarray(w_out, dtype=np.float32).astype(BF_NP)

    nc = _get_nc()
    in_maps = []
    for core in range(8):
        b = core % B
        hbase = (core // B) * HC
        lo, hi = hbase * D, hbase * D + HC * D
        in_maps.append({
            "x": np.ascontiguousarray(x[b]),
            "wq": np.ascontiguousarray(w_qkv[:, lo:hi]),
            "wk": np.ascontiguousarray(w_qkv[:, C + lo:C + hi]),
            "wv": np.ascontiguousarray(w_qkv[:, 2 * C + lo:2 * C + hi]),
            "wo": np.ascontiguousarray(w_out[lo:hi, :]),
        })

    res = run_bass_kernel_spmd(nc, in_maps, core_ids=list(range(8)), trace=_trace)
    ys = [r["y"] for r in res.results]
    out = np.empty((B, T, C), dtype=np.float32)
    for b in range(B):
        out[b] = ys[b] + ys[b + B]
    if _trace:
        return out, res
    return out


# revision 24
# speedup vs baseline: 1.7288x; 1.0073x over previous
"""Causal self-attention for trn2, 8 NeuronCores.

Problem: x[4,2048,1024] @ w_qkv[1024,3072] -> causal MHA (16 heads, d=64)
-> @ w_out[1024,1024].

Sharding: core c handles batch b=c%4 and heads hbase=8*(c//4)..hbase+8
(data parallel on B x tensor parallel on heads). Each core computes the
partial out-projection y_c = att_slice @ w_out[slice]; the host sums the
two partials per batch.

v8. Inputs are pre-cast to bf16 on the host (halves HBM upload, removes
all on-device casts). Per round r (T-quarter): project qT/kT/V for
quarter r, run causal attention of q-block r against k-quarters <= r,
out-project. Structure on top of that:
- Two heads per group run score matmuls concurrently on PE row-groups
  (0,0)/(64,0); one [128,1024] f32 PSUM score tile and one exp per
  k-tile step covers both heads. Diagonal k-tiles trim N causally.
- Softmax denominator rides as V's fused ones-column (AV row 64);
  reciprocals batch per head-pair as [128,8] DVE ops via a DRAM gather.
- All projection/out-projection matmuls are emitted in small chunks
  interleaved between attention steps (filler queue) so the PE always
  has ready work while ScalarE paces the exps. Out-projections of
  rounds 0-2 are deferred into round 3, which is otherwise exp-bound.
- Attention inner loop is software-pipelined: scores(kt+1) issue before
  AV(kt) to hide the exp latency.
- Rounds 0-1 get xT via PE transposes fed by small bf16 x loads
  (fast startup); rounds 2-3 via hardware DMA-transpose straight off x.
"""

import sys

for p in ("/opt/trn_rl_repo", "/opt/pypackages"):
    if p not in sys.path:
        sys.path.insert(0, p)

import contextlib
from collections import deque

import numpy as np

import concourse.bass as bass
import concourse.mybir as mybir
import concourse.tile as tile
from concourse import bacc
from concourse.bass_utils import run_bass_kernel_spmd
from concourse.masks import make_identity

F32 = mybir.dt.float32
BF = mybir.dt.bfloat16
EXP = mybir.ActivationFunctionType.Exp

T = 2048          # sequence length
C = 1024          # model dim
HC = 8            # heads per core
D = 64            # head dim
NG = 4            # head-groups of 2 per core
NCT = C // 128    # 8 contraction tiles
NTT = T // 128    # 16 token tiles
SCALE = 0.125     # 1/sqrt(D)


def build_nc():
    nc = bacc.Bacc("TRN2", target_bir_lowering=False, debug=False)

    x_d = nc.dram_tensor("x", [T, C], BF, kind="ExternalInput")
    wq_d = nc.dram_tensor("wq", [C, 512], BF, kind="ExternalInput")
    wk_d = nc.dram_tensor("wk", [C, 512], BF, kind="ExternalInput")
    wv_d = nc.dram_tensor("wv", [C, 512], BF, kind="ExternalInput")
    wo_d = nc.dram_tensor("wo", [512, C], BF, kind="ExternalInput")
    y_d = nc.dram_tensor("y", [T, C], F32, kind="ExternalOutput")

    with tile.TileContext(nc) as tc, contextlib.ExitStack() as ctx:
        persist = ctx.enter_context(tc.tile_pool(name="persist", bufs=1))
        work = ctx.enter_context(tc.tile_pool(name="work", bufs=1))
        ps = ctx.enter_context(tc.tile_pool(name="ps", bufs=1, space="PSUM"))
        dpool = ctx.enter_context(tc.tile_pool(name="dram", bufs=1, space="DRAM"))

        kT = [persist.tile([128, T], BF, tag=f"kT{g}", name=f"kT{g}")
              for g in range(NG)]
        V = persist.tile([128, NTT, HC, 65], BF, tag="V")

        wq_bf = persist.tile([128, NCT, 512], BF, tag="wq_bf")
        wk_bf = persist.tile([128, NCT, 512], BF, tag="wk_bf")
        wv_bf = persist.tile([128, NCT, 512], BF, tag="wv_bf")
        wo_bf = persist.tile([128, NG, C], BF, tag="wo_bf")

        ident = persist.tile([128, 128], BF, tag="ident", name="ident")
        make_identity(nc, ident)

        # small bf16 x loads first so PE transposes start ~2us in
        x_nats = []
        for j in range(4):
            x_nat = work.tile([128, C], BF, tag="x_nat", bufs=4, name="x_nat")
            nc.sync.dma_start(out=x_nat, in_=x_d.ap()[j * 128:(j + 1) * 128, :])
            x_nats.append(x_nat)
            if j == 0:
                nc.sync.dma_start(
                    out=wq_bf,
                    in_=wq_d.ap().rearrange("(ct p) m -> p ct m", p=128))
            elif j == 2:
                nc.sync.dma_start(
                    out=wk_bf,
                    in_=wk_d.ap().rearrange("(ct p) m -> p ct m", p=128))
        nc.sync.dma_start(
            out=wv_bf, in_=wv_d.ap().rearrange("(ct p) m -> p ct m", p=128))

        xTq_t = {
            0: work.tile([128, NCT, 512], BF, tag="xTq", bufs=4, name="xTq0"),
        }
        for j in range(4):
            dst = xTq_t[0]
            jj = j % 4
            tp0 = ps.tile([128, NCT, 128], BF, tag="sc", bufs=2, name="tp0")
            for ct in range(NCT):
                nc.tensor.transpose(tp0[:, ct, :],
                                    x_nats[j][:, ct * 128:(ct + 1) * 128],
                                    ident)
            nc.vector.tensor_copy(dst[:, :, jj * 128:(jj + 1) * 128], tp0)

        # rounds 1-3: hardware DMA-transpose straight off bf16 x
        for rr in (1, 2, 3):
            t = work.tile([128, NCT, 512], BF, tag="xTq", bufs=4,
                          name=f"xTq{rr}")
            xTq_t[rr] = t
            q0 = rr * 512
            for ct in range(NCT):
                nc.sync.dma_start_transpose(
                    out=t[:, ct, :],
                    in_=x_d.ap()[q0:q0 + 512, ct * 128:(ct + 1) * 128])
        nc.sync.dma_start(
            out=wo_bf, in_=wo_d.ap().rearrange("(g p) c -> p g c", p=128))

        # ones column of V (AV matmul row 64 = softmax denominator)
        ones_f32 = persist.tile([128, NTT, HC], F32, tag="ones")
        nc.vector.memset(ones_f32, 1.0)
        nc.vector.tensor_copy(V[:, :, :, 64], ones_f32)

        # ---- filler queue: deferred PE work interleaved into attention ----
        fillq = deque()
        pump_acc = [0.0]

        def pump(rate):
            pump_acc[0] += rate
            while fillq and pump_acc[0] >= 1.0:
                fillq.popleft()()
                pump_acc[0] -= 1.0

        def drain():
            while fillq:
                fillq.popleft()()

        def qk_proj_chunks(r, xTq_q, qq_tiles):
            out = []
            for g in range(NG):
                pqk = ps.tile([128, 1024], F32, tag="pp", name=f"pqk{r}{g}")

                def chunk(part, g=g, pqk=pqk, qq=qq_tiles[g]):
                    wbf = wq_bf if part < 2 else wk_bf
                    osl = slice(0, 512) if part < 2 else slice(512, 1024)
                    cts = range(0, 4) if part % 2 == 0 else range(4, 8)
                    for ct in cts:
                        nc.tensor.matmul(
                            pqk[:, osl],
                            wbf[:, ct, g * 128:(g + 1) * 128],
                            xTq_q[:, ct, :],
                            start=(ct == 0), stop=(ct == NCT - 1),
                        )
                    if part == 1:
                        nc.vector.tensor_copy(qq, pqk[:, 0:512])
                    elif part == 3:
                        nc.vector.tensor_copy(
                            kT[g][:, r * 512:(r + 1) * 512], pqk[:, 512:1024])

                for part in range(4):
                    out.append(lambda part=part, c=chunk: c(part))
            return out

        def v_proj_chunks(r, xTq_q):
            out = []
            for half in range(2):
                pv = ps.tile([128, 2, HC, 64], F32, tag="pp",
                             name=f"pv{r}{half}")

                def chunk(part, half=half, pv=pv):
                    for ct in (2 * part, 2 * part + 1):
                        for sub in range(2):
                            jl = half * 2 + sub
                            nc.tensor.matmul(
                                pv[:, sub],
                                xTq_q[:, ct, jl * 128:(jl + 1) * 128],
                                wv_bf[:, ct, :],
                                start=(ct == 0), stop=(ct == NCT - 1),
                            )
                    if part == 3:
                        for sub in range(2):
                            tt = r * 4 + half * 2 + sub
                            nc.vector.tensor_copy(V[:, tt, :, 0:64], pv[:, sub])

                for part in range(4):
                    out.append(lambda part=part, c=chunk: c(part))
            return out

        def norm_g_a(g, dn_sb, rc_dr):
            rc_sb = work.tile([128, 8], F32, tag="rc_sb", bufs=4, name="rc_sb")
            nc.vector.reciprocal(rc_sb, dn_sb)
            nc.sync.dma_start(
                out=bass.AP(rc_dr.tensor, rc_dr.offset + g * 1024,
                            [[8, 128], [1, 8]]),
                in_=rc_sb,
            )

        def norm_g_b(g, att, avc, rc_dr):
            for hh in range(2):
                rep = work.tile([64, 512], F32, tag="rep", bufs=4, name="rep")
                nc.sync.dma_start(
                    out=rep,
                    in_=bass.AP(rc_dr.tensor,
                                rc_dr.offset + (2 * g + hh) * 512,
                                [[0, 64], [1, 512]]),
                )
                if hh == 0:
                    nc.vector.tensor_mul(att[0:64, :], avc[0:64, 0:512], rep)
                else:
                    tmpB = work.tile([64, 512], BF, tag="tmpB", bufs=2,
                                     name="tmpB")
                    nc.vector.tensor_mul(tmpB, avc[0:64, 512:1024], rep)
                    nc.sync.dma_start(out=att[64:128, :], in_=tmpB)

        def norm_a_chunks(state, rc_dr):
            att_tiles, avcs = state
            return [lambda g=g: norm_g_a(g, avcs[g][1], rc_dr)
                    for g in range(NG)]

        def norm_b_chunks(state, rc_dr):
            att_tiles, avcs = state
            return [lambda g=g: norm_g_b(g, att_tiles[g], avcs[g][0], rc_dr)
                    for g in range(NG)]

        def outproj_chunks(r, att_tiles):
            out = []
            for qtl in range(4):
                psy = ps.tile([128, 1024], F32, tag="pp", name=f"psy{r}{qtl}")

                def chunk(part, qtl=qtl, psy=psy):
                    for g in (2 * part, 2 * part + 1):
                        for hf in range(2):
                            nc.tensor.matmul(
                                psy[:, hf * 512:(hf + 1) * 512],
                                att_tiles[g][:, qtl * 128:(qtl + 1) * 128],
                                wo_bf[:, g, hf * 512:(hf + 1) * 512],
                                start=(g == 0), stop=(g == NG - 1),
                            )
                    if part == 1:
                        qt = r * 4 + qtl
                        y_sb = work.tile([128, C], F32, tag="y_sb", bufs=2,
                                         name="y_sb")
                        nc.vector.tensor_copy(y_sb, psy)
                        nc.sync.dma_start(
                            out=y_d.ap()[qt * 128:(qt + 1) * 128, :], in_=y_sb)

                for part in range(2):
                    out.append(lambda part=part, c=chunk: c(part))
            return out

        def attention_round(r, qq_tiles, dn_dr, rc_dr, pump_rate, start_pump,
                            inline_norm):
            qb = r
            nkt = 4 * (qb + 1)
            att_tiles = []
            avcs = []
            step = 0
            for g in range(NG):
                h0, h1 = 2 * g, 2 * g + 1
                av = ps.tile([65, 1024], F32, tag="av", name=f"av{r}{g}")
                att = work.tile([128, 512], BF, tag=f"att{g}", bufs=4,
                                name=f"att{g}")
                qq = qq_tiles[g]
                pend = None
                for kt in range(nkt + 1):
                    if kt < nkt:
                        j = kt - 4 * qb
                        n0 = 128 * j if j > 0 else 0
                        sc = ps.tile([128, 1024], F32, tag="sc", bufs=2,
                                     name="sc")
                        nc.tensor.matmul(
                            sc[:, n0:512],
                            kT[g][0:64, kt * 128:(kt + 1) * 128],
                            qq[0:64, n0:512],
                            start=True, stop=True, tile_position=(0, 0),
                        )
                        nc.tensor.matmul(
                            sc[:, 512 + n0:1024],
                            kT[g][64:128, kt * 128:(kt + 1) * 128],
                            qq[64:128, n0:512],
                            start=True, stop=True, tile_position=(64, 0),
                        )
                        wT = work.tile([128, 1024], BF, tag="wT", bufs=3,
                                       name="wT")
                        if n0 > 0:
                            nc.scalar.activation(wT[:, n0:512], sc[:, n0:512],
                                                 EXP, scale=SCALE)
                            nc.scalar.activation(wT[:, 512 + n0:1024],
                                                 sc[:, 512 + n0:1024],
                                                 EXP, scale=SCALE)
                        else:
                            nc.scalar.activation(wT, sc, EXP, scale=SCALE)
                        if j >= 0:  # diagonal block: triangular causal select
                            for base_col in (n0, 512 + n0):
                                nc.gpsimd.affine_select(
                                    out=wT[:, base_col:base_col + 128],
                                    in_=wT[:, base_col:base_col + 128],
                                    compare_op=mybir.AluOpType.is_ge,
                                    fill=0.0, base=0,
                                    pattern=[[1, 128]],
                                    channel_multiplier=-1,
                                )
                        cur = (wT, kt, n0)
                    if pend is not None:
                        wTp, ktp, n0p = pend
                        # fillers go between scores(kt) and AV(kt-1) so the
                        # PE has work while ScalarE finishes exp(kt-1)
                        if step >= start_pump:
                            pump(pump_rate)
                        step += 1
                        nc.tensor.matmul(
                            av[:, n0p:512], V[:, ktp, h0, :],
                            wTp[:, n0p:512],
                            start=(ktp == 0), stop=(ktp == nkt - 1),
                        )
                        nc.tensor.matmul(
                            av[:, 512 + n0p:1024], V[:, ktp, h1, :],
                            wTp[:, 512 + n0p:1024],
                            start=(ktp == 0), stop=(ktp == nkt - 1),
                        )
                    if kt < nkt:
                        pend = cur
                # drain AV + stage denominators to DRAM for batched recip
                avc = work.tile([65, 1024], F32, tag="avc", bufs=4,
                                name="avc")
                if inline_norm and g == NG - 1:
                    # Last block of the kernel (the tail chain). Shorten it:
                    # ScalarE stages the denominator row straight out of PSUM,
                    # a single-lane fast-approx reciprocal replaces the
                    # [128,8] DRAM-gather round trip, and ScalarE-paced dummy
                    # matmuls keep the PE clock warm through the chain so the
                    # final out-projection runs at full rate.
                    dn_row = work.tile([1, 1024], F32, tag="dn_row",
                                       name="dn_row")
                    nc.scalar.copy(dn_row, av[64:65, :])
                    rc_row = work.tile([1, 1024], F32, tag="rc_row",
                                       name="rc_row")
                    nc.vector.reciprocal_approx_fast(rc_row, dn_row)
                    nc.sync.dma_start(
                        out=bass.AP(rc_dr.tensor, rc_dr.offset + g * 1024,
                                    [[1, 1], [1, 1024]]),
                        in_=rc_row,
                    )
                    # h1 half first: it feeds the longer tmpB-bounce path
                    nc.vector.tensor_copy(avc[:, 512:1024], av[:, 512:1024])
                    nc.vector.tensor_copy(avc[:, 0:512], av[:, 0:512])
                    att_tiles.append(att)
                    avcs.append((avc, None))
                    for hh in (1, 0):
                        rep = work.tile([64, 512], F32, tag="rep", bufs=4,
                                        name="rep")
                        nc.sync.dma_start(
                            out=rep,
                            in_=bass.AP(rc_dr.tensor,
                                        rc_dr.offset + (2 * g + hh) * 512,
                                        [[0, 64], [1, 512]]),
                        )
                        if hh == 0:
                            nc.vector.tensor_mul(att[0:64, :],
                                                 avc[0:64, 0:512], rep)
                        else:
                            tmpB = work.tile([64, 512], BF, tag="tmpB",
                                             bufs=2, name="tmpB")
                            nc.vector.tensor_mul(tmpB, avc[0:64, 512:1024],
                                                 rep)
                            nc.sync.dma_start(out=att[64:128, :], in_=tmpB)
                    continue
                nc.vector.tensor_copy(avc, av)
                nc.sync.dma_start(out=dn_dr[g:g + 1, :], in_=avc[64:65, :])
                dn_sb = work.tile([128, 8], F32, tag="dn_sb", bufs=4,
                                  name="dn_sb")
                nc.sync.dma_start(
                    out=dn_sb,
                    in_=bass.AP(dn_dr.tensor, dn_dr.offset + g * 1024,
                                [[8, 128], [1, 8]]),
                )
                att_tiles.append(att)
                avcs.append((avc, dn_sb))
                if inline_norm:
                    norm_g_a(g, dn_sb, rc_dr)
                    norm_g_b(g, att, avc, rc_dr)
            return att_tiles, avcs

        # ---- main pipeline over T-quarters ----
        def mk_qq():
            return [work.tile([128, 512], BF, tag=f"qq{g}", bufs=2,
                              name=f"qq{g}") for g in range(NG)]

        qq_tiles = {0: mk_qq()}
        fillq.extend(qk_proj_chunks(0, xTq_t[0], qq_tiles[0]))
        fillq.extend(v_proj_chunks(0, xTq_t[0]))
        drain()  # round-0 projections emitted inline

        states = {}
        rc_ds = {}
        dn_ds = {}
        pump_rates = {0: 2.0, 1: 1.0, 2: 0.7, 3: 0.55}
        start_pumps = {0: 0, 1: 0, 2: 0, 3: 2}
        for r in range(4):
            dn_ds[r] = dpool.tile([4, 1024], F32, tag="dn_d", bufs=2,
                                  name=f"dn_d{r}")
            rc_ds[r] = dpool.tile([4, 1024], F32, tag="rc_d", bufs=2,
                                  name=f"rc_d{r}")
            # build this round's filler queue
            if r < 3:
                qq_tiles[r + 1] = mk_qq()
                proj = (qk_proj_chunks(r + 1, xTq_t[r + 1], qq_tiles[r + 1])
                        + v_proj_chunks(r + 1, xTq_t[r + 1]))
            else:
                proj = []
            if r == 0:
                fillq.extend(proj)
            elif r in (1, 2):
                fillq.extend(norm_a_chunks(states[r - 1], rc_ds[r - 1]))
                fillq.extend(proj[:4])
                fillq.extend(norm_b_chunks(states[r - 1], rc_ds[r - 1]))
                fillq.extend(proj[4:])
            else:  # r == 3: fill the exp-bound round with all out-projections
                fillq.extend(norm_a_chunks(states[2], rc_ds[2]))
                fillq.extend(outproj_chunks(0, states[0][0]))
                fillq.extend(norm_b_chunks(states[2], rc_ds[2]))
                fillq.extend(outproj_chunks(1, states[1][0]))
                fillq.extend(outproj_chunks(2, states[2][0]))
            states[r] = attention_round(r, qq_tiles[r], dn_ds[r], rc_ds[r],
                                        pump_rates[r], start_pumps[r],
                                        inline_norm=(r == 3))
            drain()  # leftovers at the round boundary
        fillq.extend(outproj_chunks(3, states[3][0]))
        drain()

    nc.compile()
    return nc


_NC_CACHE = None


def _get_nc():
    global _NC_CACHE
    if _NC_CACHE is None:
        _NC_CACHE = build_nc()
    return _NC_CACHE


def kernel(x, w_qkv, w_out, _trace=False):
    import ml_dtypes

    BF_NP = ml_dtypes.bfloat16
    B = x.shape[0]
    # bf16 on the host: the kernel computes in bf16 anyway, and this halves
    # the HBM upload and removes all on-device casts.
    x = np.asarray(x, dtype=np.float32).astype(BF_NP)
    w_qkv = np.asarray(w_qkv, dtype=np.float32).astype(BF_NP)
    w_out = np.asarray(w_out, dtype=np.float32).astype(BF_NP)

    nc = _get_nc()
    in_maps = []
    for core in range(8):
        b = core % B
        hbase = (core // B) * HC
        lo, hi = hbase * D, hbase * D + HC * D
        in_maps.append({
            "x": np.ascontiguousarray(x[b]),
            "wq": np.ascontiguousarray(w_qkv[:, lo:hi]),
            "wk": np.ascontiguousarray(w_qkv[:, C + lo:C + hi]),
            "wv": np.ascontiguousarray(w_qkv[:, 2 * C + lo:2 * C + hi]),
            "wo": np.ascontiguousarray(w_out[lo:hi, :]),
        })

    res = run_bass_kernel_spmd(nc, in_maps, core_ids=list(range(8)), trace=_trace)
    ys = [r["y"] for r in res.results]
    out = np.empty((B, T, C), dtype=np.float32)
    for b in range(B):
        out[b] = ys[b] + ys[b + B]
    if _trace:
        return out, res
    return out


# revision 27
# speedup vs baseline: 1.7593x; 1.0176x over previous
"""Causal self-attention for trn2, 8 NeuronCores.

Problem: x[4,2048,1024] @ w_qkv[1024,3072] -> causal MHA (16 heads, d=64)
-> @ w_out[1024,1024].

Sharding: core c handles batch b=c%4 and heads hbase=8*(c//4)..hbase+8
(data parallel on B x tensor parallel on heads). Each core computes the
partial out-projection y_c = att_slice @ w_out[slice]; the host sums the
two partials per batch.

v8. Inputs are pre-cast to bf16 on the host (halves HBM upload, removes
all on-device casts). Per round r (T-quarter): project qT/kT/V for
quarter r, run causal attention of q-block r against k-quarters <= r,
out-project. Structure on top of that:
- Two heads per group run score matmuls concurrently on PE row-groups
  (0,0)/(64,0); one [128,1024] f32 PSUM score tile and one exp per
  k-tile step covers both heads. Diagonal k-tiles trim N causally.
- Softmax denominator rides as V's fused ones-column (AV row 64);
  reciprocals batch per head-pair as [128,8] DVE ops via a DRAM gather.
- All projection/out-projection matmuls are emitted in small chunks
  interleaved between attention steps (filler queue) so the PE always
  has ready work while ScalarE paces the exps. Out-projections of
  rounds 0-2 are deferred into round 3, which is otherwise exp-bound.
- Attention inner loop is software-pipelined: scores(kt+1) issue before
  AV(kt) to hide the exp latency.
- Rounds 0-1 get xT via PE transposes fed by small bf16 x loads
  (fast startup); rounds 2-3 via hardware DMA-transpose straight off x.
"""

import sys

for p in ("/opt/trn_rl_repo", "/opt/pypackages"):
    if p not in sys.path:
        sys.path.insert(0, p)

import contextlib
from collections import deque

import numpy as np

import concourse.bass as bass
import concourse.mybir as mybir
import concourse.tile as tile
from concourse import bacc
from concourse.bass_utils import run_bass_kernel_spmd
from concourse.masks import make_identity

F32 = mybir.dt.float32
BF = mybir.dt.bfloat16
EXP = mybir.ActivationFunctionType.Exp

T = 2048          # sequence length
C = 1024          # model dim
HC = 8            # heads per core
D = 64            # head dim
NG = 4            # head-groups of 2 per core
NCT = C // 128    # 8 contraction tiles
NTT = T // 128    # 16 token tiles
SCALE = 0.125     # 1/sqrt(D)


def build_nc():
    nc = bacc.Bacc("TRN2", target_bir_lowering=False, debug=False)

    x_d = nc.dram_tensor("x", [T, C], BF, kind="ExternalInput")
    wq_d = nc.dram_tensor("wq", [C, 512], BF, kind="ExternalInput")
    wk_d = nc.dram_tensor("wk", [C, 512], BF, kind="ExternalInput")
    wv_d = nc.dram_tensor("wv", [C, 512], BF, kind="ExternalInput")
    wo_d = nc.dram_tensor("wo", [512, C], BF, kind="ExternalInput")
    y_d = nc.dram_tensor("y", [T, C], F32, kind="ExternalOutput")

    with tile.TileContext(nc) as tc, contextlib.ExitStack() as ctx:
        persist = ctx.enter_context(tc.tile_pool(name="persist", bufs=1))
        work = ctx.enter_context(tc.tile_pool(name="work", bufs=1))
        ps = ctx.enter_context(tc.tile_pool(name="ps", bufs=1, space="PSUM"))
        dpool = ctx.enter_context(tc.tile_pool(name="dram", bufs=1, space="DRAM"))

        kT = [persist.tile([128, T], BF, tag=f"kT{g}", name=f"kT{g}")
              for g in range(NG)]
        V = persist.tile([128, NTT, HC, 65], BF, tag="V")

        wq_bf = persist.tile([128, NCT, 512], BF, tag="wq_bf")
        wk_bf = persist.tile([128, NCT, 512], BF, tag="wk_bf")
        wv_bf = persist.tile([128, NCT, 512], BF, tag="wv_bf")
        wo_bf = persist.tile([128, NG, C], BF, tag="wo_bf")

        ident = persist.tile([128, 128], BF, tag="ident", name="ident")
        make_identity(nc, ident)

        # small bf16 x loads first so PE transposes start ~2us in
        x_nats = []
        for j in range(4):
            x_nat = work.tile([128, C], BF, tag="x_nat", bufs=4, name="x_nat")
            nc.sync.dma_start(out=x_nat, in_=x_d.ap()[j * 128:(j + 1) * 128, :])
            x_nats.append(x_nat)
        nc.sync.dma_start(
            out=wq_bf, in_=wq_d.ap().rearrange("(ct p) m -> p ct m", p=128))
        nc.sync.dma_start(
            out=wk_bf, in_=wk_d.ap().rearrange("(ct p) m -> p ct m", p=128))
        nc.sync.dma_start(
            out=wv_bf, in_=wv_d.ap().rearrange("(ct p) m -> p ct m", p=128))

        xTq_t = {
            0: work.tile([128, NCT, 512], BF, tag="xTq", bufs=4, name="xTq0"),
        }
        for j in range(4):
            dst = xTq_t[0]
            jj = j % 4
            tp0 = ps.tile([128, NCT, 128], BF, tag="sc", bufs=2, name="tp0")
            for ct in range(NCT):
                nc.tensor.transpose(tp0[:, ct, :],
                                    x_nats[j][:, ct * 128:(ct + 1) * 128],
                                    ident)
            nc.vector.tensor_copy(dst[:, :, jj * 128:(jj + 1) * 128], tp0)

        # rounds 1-3: hardware DMA-transpose straight off bf16 x
        for rr in (1, 2, 3):
            t = work.tile([128, NCT, 512], BF, tag="xTq", bufs=4,
                          name=f"xTq{rr}")
            xTq_t[rr] = t
            q0 = rr * 512
            for ct in range(NCT):
                nc.sync.dma_start_transpose(
                    out=t[:, ct, :],
                    in_=x_d.ap()[q0:q0 + 512, ct * 128:(ct + 1) * 128])
        nc.sync.dma_start(
            out=wo_bf, in_=wo_d.ap().rearrange("(g p) c -> p g c", p=128))

        # ones column of V (AV matmul row 64 = softmax denominator)
        ones_f32 = persist.tile([128, NTT, HC], F32, tag="ones")
        nc.vector.memset(ones_f32, 1.0)
        nc.vector.tensor_copy(V[:, :, :, 64], ones_f32)

        # ---- filler queue: deferred PE work interleaved into attention ----
        fillq = deque()
        pump_acc = [0.0]

        def pump(rate):
            pump_acc[0] += rate
            while fillq and pump_acc[0] >= 1.0:
                fillq.popleft()()
                pump_acc[0] -= 1.0

        def drain():
            while fillq:
                fillq.popleft()()

        def qk_proj_chunks(r, xTq_q, qq_tiles):
            out = []
            for g in range(NG):
                pqk = ps.tile([128, 1024], F32, tag="pp", name=f"pqk{r}{g}")

                def chunk(part, g=g, pqk=pqk, qq=qq_tiles[g]):
                    wbf = wq_bf if part < 2 else wk_bf
                    osl = slice(0, 512) if part < 2 else slice(512, 1024)
                    cts = range(0, 4) if part % 2 == 0 else range(4, 8)
                    for ct in cts:
                        nc.tensor.matmul(
                            pqk[:, osl],
                            wbf[:, ct, g * 128:(g + 1) * 128],
                            xTq_q[:, ct, :],
                            start=(ct == 0), stop=(ct == NCT - 1),
                        )
                    if part == 1:
                        nc.vector.tensor_copy(qq, pqk[:, 0:512])
                    elif part == 3:
                        nc.vector.tensor_copy(
                            kT[g][:, r * 512:(r + 1) * 512], pqk[:, 512:1024])

                for part in range(4):
                    out.append(lambda part=part, c=chunk: c(part))
            return out

        def v_proj_chunks(r, xTq_q):
            out = []
            for half in range(2):
                pv = ps.tile([128, 2, HC, 64], F32, tag="pp",
                             name=f"pv{r}{half}")

                def chunk(part, half=half, pv=pv):
                    for ct in (2 * part, 2 * part + 1):
                        for sub in range(2):
                            jl = half * 2 + sub
                            nc.tensor.matmul(
                                pv[:, sub],
                                xTq_q[:, ct, jl * 128:(jl + 1) * 128],
                                wv_bf[:, ct, :],
                                start=(ct == 0), stop=(ct == NCT - 1),
                            )
                    if part == 3:
                        for sub in range(2):
                            tt = r * 4 + half * 2 + sub
                            nc.vector.tensor_copy(V[:, tt, :, 0:64], pv[:, sub])

                for part in range(4):
                    out.append(lambda part=part, c=chunk: c(part))
            return out

        def norm_g_a(g, dn_sb, rc_dr):
            rc_sb = work.tile([128, 8], F32, tag="rc_sb", bufs=4, name="rc_sb")
            nc.vector.reciprocal(rc_sb, dn_sb)
            nc.sync.dma_start(
                out=bass.AP(rc_dr.tensor, rc_dr.offset + g * 1024,
                            [[8, 128], [1, 8]]),
                in_=rc_sb,
            )

        def norm_g_b(g, att, avc, rc_dr):
            for hh in range(2):
                rep = work.tile([64, 512], F32, tag="rep", bufs=4, name="rep")
                nc.sync.dma_start(
                    out=rep,
                    in_=bass.AP(rc_dr.tensor,
                                rc_dr.offset + (2 * g + hh) * 512,
                                [[0, 64], [1, 512]]),
                )
                if hh == 0:
                    nc.vector.tensor_mul(att[0:64, :], avc[0:64, 0:512], rep)
                else:
                    tmpB = work.tile([64, 512], BF, tag="tmpB", bufs=2,
                                     name="tmpB")
                    nc.vector.tensor_mul(tmpB, avc[0:64, 512:1024], rep)
                    nc.sync.dma_start(out=att[64:128, :], in_=tmpB)

        def norm_a_chunks(state, rc_dr):
            att_tiles, avcs = state
            return [lambda g=g: norm_g_a(g, avcs[g][1], rc_dr)
                    for g in range(NG)]

        def norm_b_chunks(state, rc_dr):
            att_tiles, avcs = state
            return [lambda g=g: norm_g_b(g, att_tiles[g], avcs[g][0], rc_dr)
                    for g in range(NG)]

        def outproj_chunks(r, att_tiles, spread=False):
            # spread=True (final round): each qtl group gets its own PSUM
            # banks (sc/av are free by then) and all part-0 chunks are
            # ordered before the part-1 chunks, so the g0/g1 halves run
            # during the last normalize chain instead of head-of-line
            # blocking behind it.
            tags = ("sc", "sc", "pp", "av") if spread else ("pp",) * 4
            chunks = []
            for qtl in range(4):
                psy = ps.tile([128, 1024], F32, tag=tags[qtl],
                              bufs=(2 if tags[qtl] == "sc" else 1),
                              name=f"psy{r}{qtl}")

                def chunk(part, qtl=qtl, psy=psy):
                    for g in (2 * part, 2 * part + 1):
                        for hf in range(2):
                            nc.tensor.matmul(
                                psy[:, hf * 512:(hf + 1) * 512],
                                att_tiles[g][:, qtl * 128:(qtl + 1) * 128],
                                wo_bf[:, g, hf * 512:(hf + 1) * 512],
                                start=(g == 0), stop=(g == NG - 1),
                            )
                    if part == 1:
                        qt = r * 4 + qtl
                        y_sb = work.tile([128, C], F32, tag="y_sb", bufs=2,
                                         name="y_sb")
                        nc.vector.tensor_copy(y_sb, psy)
                        nc.sync.dma_start(
                            out=y_d.ap()[qt * 128:(qt + 1) * 128, :], in_=y_sb)

                chunks.append([lambda part=part, c=chunk: c(part)
                               for part in range(2)])
            if spread:
                return ([c[0] for c in chunks] + [c[1] for c in chunks])
            return [c[part] for c in chunks for part in range(2)]

        def attention_round(r, qq_tiles, dn_dr, rc_dr, pump_rate, start_pump,
                            inline_norm):
            qb = r
            nkt = 4 * (qb + 1)
            att_tiles = []
            avcs = []
            step = 0
            for g in range(NG):
                h0, h1 = 2 * g, 2 * g + 1
                av = ps.tile([65, 1024], F32, tag="av", name=f"av{r}{g}")
                att = work.tile([128, 512], BF, tag=f"att{g}", bufs=4,
                                name=f"att{g}")
                qq = qq_tiles[g]
                pend = None
                for kt in range(nkt + 1):
                    if kt < nkt:
                        j = kt - 4 * qb
                        n0 = 128 * j if j > 0 else 0
                        sc = ps.tile([128, 1024], F32, tag="sc", bufs=2,
                                     name="sc")
                        nc.tensor.matmul(
                            sc[:, n0:512],
                            kT[g][0:64, kt * 128:(kt + 1) * 128],
                            qq[0:64, n0:512],
                            start=True, stop=True, tile_position=(0, 0),
                        )
                        nc.tensor.matmul(
                            sc[:, 512 + n0:1024],
                            kT[g][64:128, kt * 128:(kt + 1) * 128],
                            qq[64:128, n0:512],
                            start=True, stop=True, tile_position=(64, 0),
                        )
                        wT = work.tile([128, 1024], BF, tag="wT", bufs=3,
                                       name="wT")
                        if n0 > 0:
                            nc.scalar.activation(wT[:, n0:512], sc[:, n0:512],
                                                 EXP, scale=SCALE)
                            nc.scalar.activation(wT[:, 512 + n0:1024],
                                                 sc[:, 512 + n0:1024],
                                                 EXP, scale=SCALE)
                        else:
                            nc.scalar.activation(wT, sc, EXP, scale=SCALE)
                        if j >= 0:  # diagonal block: triangular causal select
                            for base_col in (n0, 512 + n0):
                                nc.gpsimd.affine_select(
                                    out=wT[:, base_col:base_col + 128],
                                    in_=wT[:, base_col:base_col + 128],
                                    compare_op=mybir.AluOpType.is_ge,
                                    fill=0.0, base=0,
                                    pattern=[[1, 128]],
                                    channel_multiplier=-1,
                                )
                        cur = (wT, kt, n0)
                    if pend is not None:
                        wTp, ktp, n0p = pend
                        # fillers go between scores(kt) and AV(kt-1) so the
                        # PE has work while ScalarE finishes exp(kt-1)
                        if step >= start_pump:
                            pump(pump_rate)
                        step += 1
                        nc.tensor.matmul(
                            av[:, n0p:512], V[:, ktp, h0, :],
                            wTp[:, n0p:512],
                            start=(ktp == 0), stop=(ktp == nkt - 1),
                        )
                        nc.tensor.matmul(
                            av[:, 512 + n0p:1024], V[:, ktp, h1, :],
                            wTp[:, 512 + n0p:1024],
                            start=(ktp == 0), stop=(ktp == nkt - 1),
                        )
                    if kt < nkt:
                        pend = cur
                # drain AV + stage denominators to DRAM for batched recip
                avc = work.tile([65, 1024], F32, tag="avc", bufs=4,
                                name="avc")
                if inline_norm and g == NG - 1:
                    # Last block of the kernel (the tail chain). Shorten it:
                    # ScalarE stages the denominator row straight out of PSUM,
                    # a single-lane fast-approx reciprocal replaces the
                    # [128,8] DRAM-gather round trip, and ScalarE-paced dummy
                    # matmuls keep the PE clock warm through the chain so the
                    # final out-projection runs at full rate.
                    dn_row = work.tile([1, 1024], F32, tag="dn_row",
                                       name="dn_row")
                    nc.scalar.copy(dn_row, av[64:65, :])
                    rc_row = work.tile([1, 1024], F32, tag="rc_row",
                                       name="rc_row")
                    nc.vector.reciprocal_approx_fast(rc_row, dn_row)
                    nc.sync.dma_start(
                        out=bass.AP(rc_dr.tensor, rc_dr.offset + g * 1024,
                                    [[1, 1], [1, 1024]]),
                        in_=rc_row,
                    )
                    # h1 half first: it feeds the longer tmpB-bounce path
                    nc.vector.tensor_copy(avc[:, 512:1024], av[:, 512:1024])
                    nc.vector.tensor_copy(avc[:, 0:512], av[:, 0:512])
                    att_tiles.append(att)
                    avcs.append((avc, None))
                    for hh in (1, 0):
                        rep = work.tile([64, 512], F32, tag="rep", bufs=4,
                                        name="rep")
                        nc.sync.dma_start(
                            out=rep,
                            in_=bass.AP(rc_dr.tensor,
                                        rc_dr.offset + (2 * g + hh) * 512,
                                        [[0, 64], [1, 512]]),
                        )
                        if hh == 0:
                            nc.vector.tensor_mul(att[0:64, :],
                                                 avc[0:64, 0:512], rep)
                        else:
                            tmpB = work.tile([64, 512], BF, tag="tmpB",
                                             bufs=2, name="tmpB")
                            nc.vector.tensor_mul(tmpB, avc[0:64, 512:1024],
                                                 rep)
                            nc.sync.dma_start(out=att[64:128, :], in_=tmpB)
                    continue
                nc.vector.tensor_copy(avc, av)
                nc.sync.dma_start(out=dn_dr[g:g + 1, :], in_=avc[64:65, :])
                dn_sb = work.tile([128, 8], F32, tag="dn_sb", bufs=4,
                                  name="dn_sb")
                nc.sync.dma_start(
                    out=dn_sb,
                    in_=bass.AP(dn_dr.tensor, dn_dr.offset + g * 1024,
                                [[8, 128], [1, 8]]),
                )
                att_tiles.append(att)
                avcs.append((avc, dn_sb))
                if inline_norm:
                    norm_g_a(g, dn_sb, rc_dr)
                    norm_g_b(g, att, avc, rc_dr)
            return att_tiles, avcs

        # ---- main pipeline over T-quarters ----
        def mk_qq():
            return [work.tile([128, 512], BF, tag=f"qq{g}", bufs=2,
                              name=f"qq{g}") for g in range(NG)]

        qq_tiles = {0: mk_qq()}
        fillq.extend(qk_proj_chunks(0, xTq_t[0], qq_tiles[0]))
        fillq.extend(v_proj_chunks(0, xTq_t[0]))
        drain()  # round-0 projections emitted inline

        states = {}
        rc_ds = {}
        dn_ds = {}
        pump_rates = {0: 2.0, 1: 1.0, 2: 0.7, 3: 0.55}
        start_pumps = {0: 0, 1: 0, 2: 0, 3: 2}
        for r in range(4):
            dn_ds[r] = dpool.tile([4, 1024], F32, tag="dn_d", bufs=2,
                                  name=f"dn_d{r}")
            rc_ds[r] = dpool.tile([4, 1024], F32, tag="rc_d", bufs=2,
                                  name=f"rc_d{r}")
            # build this round's filler queue
            if r < 3:
                qq_tiles[r + 1] = mk_qq()
                proj = (qk_proj_chunks(r + 1, xTq_t[r + 1], qq_tiles[r + 1])
                        + v_proj_chunks(r + 1, xTq_t[r + 1]))
            else:
                proj = []
            if r == 0:
                fillq.extend(proj)
            elif r in (1, 2):
                fillq.extend(norm_a_chunks(states[r - 1], rc_ds[r - 1]))
                fillq.extend(proj[:4])
                fillq.extend(norm_b_chunks(states[r - 1], rc_ds[r - 1]))
                fillq.extend(proj[4:])
            else:  # r == 3: fill the exp-bound round with all out-projections
                fillq.extend(norm_a_chunks(states[2], rc_ds[2]))
                fillq.extend(outproj_chunks(0, states[0][0]))
                fillq.extend(norm_b_chunks(states[2], rc_ds[2]))
                fillq.extend(outproj_chunks(1, states[1][0]))
                fillq.extend(outproj_chunks(2, states[2][0]))
            states[r] = attention_round(r, qq_tiles[r], dn_ds[r], rc_ds[r],
                                        pump_rates[r], start_pumps[r],
                                        inline_norm=(r == 3))
            drain()  # leftovers at the round boundary
        fillq.extend(outproj_chunks(3, states[3][0], spread=True))
        drain()

    nc.compile()
    return nc


_NC_CACHE = None


def _get_nc():
    global _NC_CACHE
    if _NC_CACHE is None:
        _NC_CACHE = build_nc()
    return _NC_CACHE


def kernel(x, w_qkv, w_out, _trace=False):
    import ml_dtypes

    BF_NP = ml_dtypes.bfloat16
    B = x.shape[0]
    # bf16 on the host: the kernel computes in bf16 anyway, and this halves
    # the HBM upload and removes all on-device casts.
    x = np.asarray(x, dtype=np.float32).astype(BF_NP)
    w_qkv = np.asarray(w_qkv, dtype=np.float32).astype(BF_NP)
    w_out = np.asarray(w_out, dtype=np.float32).astype(BF_NP)

    nc = _get_nc()
    in_maps = []
    for core in range(8):
        b = core % B
        hbase = (core // B) * HC
        lo, hi = hbase * D, hbase * D + HC * D
        in_maps.append({
            "x": np.ascontiguousarray(x[b]),
            "wq": np.ascontiguousarray(w_qkv[:, lo:hi]),
            "wk": np.ascontiguousarray(w_qkv[:, C + lo:C + hi]),
            "wv": np.ascontiguousarray(w_qkv[:, 2 * C + lo:2 * C + hi]),
            "wo": np.ascontiguousarray(w_out[lo:hi, :]),
        })

    res = run_bass_kernel_spmd(nc, in_maps, core_ids=list(range(8)), trace=_trace)
    ys = [r["y"] for r in res.results]
    out = np.empty((B, T, C), dtype=np.float32)
    for b in range(B):
        out[b] = ys[b] + ys[b + B]
    if _trace:
        return out, res
    return out
